# revision 1
# baseline (speedup 1.0000x reference)
"""Trainium2 Bass kernel for LorentzSelfAttention (B=8, L=2048, D=128, 1 head).

Sharding: data-parallel over batch — core b handles batch element b.

Per-core algorithm (one NeuronCore, L=2048, D=128, 16 row-chunks of 128):
  Inputs arrive HOST-TRANSPOSED: qT/kT/vT [D, L] so no on-device input
  transposes are needed (the contraction dim must sit on partitions).
  Phase B (projections), grouped 4 chunks at a time:
      linT chunk c: matmul(lhsT=xT[:, c*128:(c+1)*128], rhs=W^T) -> natural
      [l, dout] PSUM. Row-wise Lorentz stats (sigmoid / sum-of-squares) are
      batched into [128, 48] stat tiles so ONE ACT Sqrt serves all 48 chunks
      (Sigmoid, Sqrt, Exp live in different ACT table sets — interleaving
      them costs a ~1.3us table load per switch; Square/Copy are in every
      set and are free). q/k chunks are PE-transposed into qT/kT [d, l];
      v stays natural with padded rows zeroed.
  Phase C (attention): scores computed TRANSPOSED, S_T[j, i] = <k_j, q_i>_L
      (q time row negated), only causal columns i >= j*128, float32r matmuls.
      exp() applied without max-subtract / row-sum normalization: the final
      Lorentz mid-point normalization out = ave/sqrt(|<ave,ave>_L|) is
      scale-invariant per row, so all softmax constants cancel. Pad masking
      is folded into v (zeroed rows); causal diag-block masking is a 0/1
      upper-tri multiply. AV accumulates transposed: outT[d, i] += v_j.T @
      expT_j (PSUM, 16 steps, float32r).
  Phase D: PE-transpose outT back to natural (grouped 4 chunks / PSUM bank),
      batched row-wise Lorentz normalize (one Sqrt), DMA out.

Rows whose allowed (causal & non-pad) key set is empty produce softmax over
an all -inf row in the reference (== uniform over ALL 2048 keys). Those rows
(a ~0-2 row prefix per batch, only when the batch's first keys are padded)
are fixed up exactly on host.
"""

import os

import numpy as np

B, L, D = 8, 2048, 128
P = 128
NCHUNK = L // P   # 16
G = 4             # chunks per group
NGROUP = NCHUNK // G  # 4

_RUNNER_CACHE: dict = {}


def _bcast3(bass, ap2, inner):
    """[P, n] AP -> [P, n, inner] broadcast view (step-0 innermost)."""
    return bass.AP(tensor=ap2.tensor, offset=ap2.offset,
                   ap=[ap2.ap[0], ap2.ap[1], [0, inner]])


# ---------------------------------------------------------------- device code
def _build_program(cfg, consts):
    from contextlib import ExitStack

    import concourse.bacc as bacc
    import concourse.bass as bass
    import concourse.mybir as mybir
    import concourse.tile as tile
    from concourse import masks

    f32 = mybir.dt.float32
    f32r = mybir.dt.float32r
    AF = mybir.ActivationFunctionType
    OP = mybir.AluOpType

    es = {"q": consts["es_q"], "k": consts["es_k"], "v": consts["es_v"]}
    c1 = consts["c1"]
    has_bias = consts["has_bias"]

    def mmc(ap, kind):
        if cfg[kind] == "f32r":
            return ap.bitcast(f32r)
        return ap

    qk_dt = f32r if cfg["mm_qk"] == "f32r" else f32
    av_dt = f32r if cfg["mm_av"] == "f32r" else f32

    nc = bacc.Bacc("TRN2", target_bir_lowering=False, debug=False)

    xT_d = {}
    for nm in ("q", "k", "v"):
        xT_d[nm] = nc.dram_tensor(nm, [D, L], f32, kind="ExternalInput").ap()
    pad_d = nc.dram_tensor("pad", [L], f32, kind="ExternalInput").ap()
    wt_d = {nm: nc.dram_tensor(f"w{nm}t", [D, D], f32, kind="ExternalInput").ap()
            for nm in ("q", "k", "v")}
    bias_d = {}
    if has_bias:
        for nm in ("q", "k", "v"):
            bias_d[nm] = nc.dram_tensor(f"b{nm}", [1, D], f32,
                                        kind="ExternalInput").ap()
    out_d = nc.dram_tensor("out", [L, D], f32, kind="ExternalOutput").ap()

    TENSORS = ("q", "k", "v")

    with tile.TileContext(nc) as tc, ExitStack() as octx:
        cpool = octx.enter_context(tc.tile_pool(name="consts", bufs=1))
        ident = cpool.tile([P, P], f32)
        masks.make_identity(nc, ident[:])
        ut01 = cpool.tile([P, P], f32)
        masks.make_upper_triangular(nc, ut01[:], val=1.0, diag=True)
        w_sb = {}
        for nm in TENSORS:
            w_sb[nm] = cpool.tile([P, D], f32, name=f"w_{nm}", tag=f"w_{nm}")
            nc.sync.dma_start(out=w_sb[nm][:], in_=wt_d[nm][:, :])
        pad_sb = cpool.tile([P, NCHUNK], f32)
        nc.sync.dma_start(out=pad_sb[:], in_=pad_d.rearrange("(c p) -> p c", p=P))
        bias_sb = {}
        if has_bias:
            for nm in TENSORS:
                bt = cpool.tile([P, D], f32)
                bd = bias_d[nm]
                nc.sync.dma_start(out=bt[:], in_=bass.AP(
                    tensor=bd.tensor, offset=bd.offset, ap=[[0, P], bd.ap[1]]))
                bias_sb[nm] = bt

        # persistent activations
        qT_sb = cpool.tile([P, L], qk_dt)      # [d, l], time row negated
        kT_sb = cpool.tile([P, L], qk_dt)
        v_sb = cpool.tile([P, NCHUNK, D], av_dt)  # [l%128, chunk, d], pad-zeroed
        qk_nat = cpool.tile([P, 2, NCHUNK, D], f32)  # q/k chunks, natural
        outT_sb = cpool.tile([P, L], f32)

        # batched per-row stats: col t*16+c is chunk c of tensor t
        time_all = cpool.tile([P, 3 * NCHUNK], f32)
        ss_all = cpool.tile([P, 3 * NCHUNK], f32)
        sqs_all = cpool.tile([P, 3 * NCHUNK], f32)

        # ---------------- Phase B: projections ----------------
        with ExitStack() as ctxB:
            xin = ctxB.enter_context(tc.tile_pool(name="xin", bufs=3))
            ps_l = ctxB.enter_context(tc.tile_pool(name="ps_l", bufs=3, space="PSUM"))
            ps_q = ctxB.enter_context(tc.tile_pool(name="ps_q", bufs=2, space="PSUM"))
            misc = ctxB.enter_context(tc.tile_pool(name="misc", bufs=3))
            stat = ctxB.enter_context(tc.tile_pool(name="stat", bufs=4))

            def dest4(nm, g):
                if nm == "v":
                    return v_sb[:, g * G:(g + 1) * G, :]
                ti = TENSORS.index(nm)
                return qk_nat[:, ti, g * G:(g + 1) * G, :]

            # pass 1: matmuls + sigmoid/square stats (ACT stays in the
            # sigmoid table set: Sigmoid+Square only), narrow -> SBUF dest
            for g in range(NGROUP):
                for nm in TENSORS:
                    ti = TENSORS.index(nm)
                    sb = ti * NCHUNK + g * G   # stats col base
                    xT4 = xin.tile([P, G * P], f32, tag=f"x{nm}")
                    nc.sync.dma_start(
                        out=xT4[:], in_=xT_d[nm][:, g * G * P:(g + 1) * G * P])
                    lin4 = ps_l.tile([P, G * D], f32, tag="lin")
                    for c in range(G):
                        nc.tensor.matmul(
                            lin4[:, c * D:(c + 1) * D],
                            mmc(xT4[:, c * P:(c + 1) * P], "mm_proj"),
                            mmc(w_sb[nm][:], "mm_proj"), start=True, stop=True)
                    if has_bias:
                        lin_sb4 = misc.tile([P, G * D], f32, tag="linb")
                        nc.vector.tensor_add(
                            lin_sb4[:], lin4[:],
                            bass.AP(tensor=bias_sb[nm].tensor,
                                    offset=bias_sb[nm][:].offset,
                                    ap=[bias_sb[nm][:].ap[0], [0, G], [1, D]]))
                        src4 = lin_sb4[:].rearrange("p (c d) -> p c d", d=D)
                    else:
                        src4 = lin4[:].rearrange("p (c d) -> p c d", d=D)
                    # sigmoid of column 0 -> sig (into time_all slot, raw)
                    nc.scalar.activation(
                        time_all[:, sb:sb + G], src4[:, :, 0:1], AF.Sigmoid)
                    # sum of squares of narrow part
                    sq4 = misc.tile([P, G, D - 1], f32, tag="sq4")
                    nc.scalar.activation(sq4[:], src4[:, :, 1:D], AF.Square)
                    nc.vector.tensor_reduce(
                        ss_all[:, sb:sb + G], sq4[:], mybir.AxisListType.X,
                        OP.add)
                    # park unscaled narrow in its SBUF destination (frees PSUM)
                    nc.vector.tensor_copy(dest4(nm, g)[:, :, 1:D],
                                          src4[:, :, 1:D])

            # pass 2: batched scalar math over all 48 chunks
            #   time = sig*exp(s) + 1.1  (per-tensor scale)
            for nm in TENSORS:
                ti = TENSORS.index(nm)
                sb = ti * NCHUNK
                nc.vector.tensor_scalar(
                    out=time_all[:, sb:sb + NCHUNK],
                    in0=time_all[:, sb:sb + NCHUNK],
                    scalar1=es[nm], scalar2=1.1, op0=OP.mult, op1=OP.add)
            inv_ss = stat.tile([P, 3 * NCHUNK], f32, tag="iss")
            nc.vector.reciprocal(inv_ss[:], ss_all[:])
            t2m1 = stat.tile([P, 3 * NCHUNK], f32, tag="t2m1")
            nc.vector.tensor_mul(t2m1[:], time_all[:], time_all[:])
            nc.vector.tensor_scalar_add(out=t2m1[:], in0=t2m1[:], scalar1=-1.0)
            sval = stat.tile([P, 3 * NCHUNK], f32, tag="sval")
            nc.vector.tensor_mul(sval[:], t2m1[:], inv_ss[:])
            nc.scalar.activation(sqs_all[:], sval[:], AF.Sqrt)
            # fold pad zeroing into v scales (time + narrow) — AFTER the sqrt
            vb = 2 * NCHUNK
            nc.vector.tensor_mul(
                sqs_all[:, vb:vb + NCHUNK], sqs_all[:, vb:vb + NCHUNK], pad_sb[:])
            nc.vector.tensor_mul(
                time_all[:, vb:vb + NCHUNK], time_all[:, vb:vb + NCHUNK],
                pad_sb[:])

            # pass 3: finish chunks in place, transpose q/k into qT/kT
            for g in range(NGROUP):
                for nm in TENSORS:
                    ti = TENSORS.index(nm)
                    sb = ti * NCHUNK + g * G
                    ch4 = dest4(nm, g)
                    tsign = -1.0 if nm == "q" else 1.0
                    # signed time into col 0
                    nc.vector.tensor_scalar(
                        out=ch4[:, :, 0:1], in0=time_all[:, sb:sb + G],
                        scalar1=tsign, scalar2=0.0, op0=OP.mult, op1=OP.add)
                    # narrow scaled by sqrt(s) in place (per-chunk broadcast)
                    nc.vector.tensor_mul(
                        ch4[:, :, 1:D], ch4[:, :, 1:D],
                        _bcast3(bass, sqs_all[:, sb:sb + G], D - 1))
                    if nm != "v":
                        qkT4 = ps_q.tile([P, G * P], f32, tag="qkT")
                        for c in range(G):
                            nc.tensor.transpose(
                                qkT4[:, c * P:(c + 1) * P], ch4[:, c, :],
                                ident[:])
                        dst = qT_sb if nm == "q" else kT_sb
                        nc.vector.tensor_copy(
                            dst[:, g * G * P:(g + 1) * G * P], qkT4[:])

        # ---------------- Phase C: attention ----------------
        with ExitStack() as ctxC:
            ps_s = ctxC.enter_context(tc.tile_pool(name="ps_s", bufs=2, space="PSUM"))
            ps_o = ctxC.enter_context(tc.tile_pool(name="ps_o", bufs=1, space="PSUM"))
            sb_e = ctxC.enter_context(tc.tile_pool(name="sb_e", bufs=2))
            outT_ps = ps_o.tile([P, L], f32)

            for j in range(NCHUNK):
                ncols = (NCHUNK - j) * P
                base = j * P
                expT = sb_e.tile([P, L], av_dt, tag="expT")  # col0 == global i=base
                kblk = kT_sb[:, base:base + P]
                ofs = 0
                while ofs < ncols:   # scores + exp in <=1024-col slabs
                    sw = min(1024, ncols - ofs)
                    s_ps = ps_s.tile([P, 1024], f32, tag="s")
                    mofs = 0
                    while mofs < sw:  # matmul N<=512 per PSUM bank
                        w = min(512, sw - mofs)
                        nc.tensor.matmul(
                            s_ps[:, mofs:mofs + w], kblk,
                            qT_sb[:, base + ofs + mofs:base + ofs + mofs + w],
                            start=True, stop=True)
                        mofs += w
                    nc.scalar.activation(
                        expT[:, ofs:ofs + sw], s_ps[:, :sw], AF.Exp, scale=c1)
                    ofs += sw
                # causal mask inside the diagonal block
                nc.vector.tensor_mul(expT[:, 0:P], expT[:, 0:P], ut01[:])
                # outT[d, i] += v_j.T @ expT_j  (bank-aligned psum chunks)
                col = base
                while col < L:
                    bank_end = min(L, (col // 512 + 1) * 512)
                    kbank = bank_end // 512 - 1
                    last_j = 4 * kbank + 3
                    nc.tensor.matmul(
                        outT_ps[:, col:bank_end],
                        v_sb[:, j, :],
                        expT[:, col - base:bank_end - base],
                        start=(j == 0), stop=(j == last_j))
                    col = bank_end

            nc.vector.tensor_copy(outT_sb[:], outT_ps[:])

        # ---------------- Phase D: normalize + store ----------------
        with ExitStack() as ctxD:
            ps_d = ctxD.enter_context(tc.tile_pool(name="ps_d", bufs=4, space="PSUM"))
            dmisc = ctxD.enter_context(tc.tile_pool(name="dmisc", bufs=3))
            dstat = ctxD.enter_context(tc.tile_pool(name="dstat", bufs=2))
            na_all = dstat.tile([P, NCHUNK], f32, tag="na")
            rn_all = dstat.tile([P, NCHUNK], f32, tag="rn")
            o_keep = {}
            for g in range(NGROUP):
                o_ps4 = ps_d.tile([P, G, D], f32, tag="o")
                o_keep[g] = o_ps4
                for c in range(G):
                    nc.tensor.transpose(
                        o_ps4[:, c, :],
                        outT_sb[:, (g * G + c) * P:(g * G + c + 1) * P],
                        ident[:])
                scr4 = dmisc.tile([P, G, D], f32, tag="scr")
                nc.scalar.activation(scr4[:], o_ps4[:], AF.Square)
                # ssum (t^2 + |n|^2) then na = -lor = 2*t^2 - ssum
                nc.vector.tensor_reduce(
                    na_all[:, g * G:(g + 1) * G], scr4[:],
                    mybir.AxisListType.X, OP.add)
                nc.vector.tensor_scalar(
                    out=scr4[:, :, 0:1], in0=scr4[:, :, 0:1], scalar1=2.0,
                    scalar2=0.0, op0=OP.mult, op1=OP.add)
                nc.vector.tensor_sub(
                    na_all[:, g * G:(g + 1) * G], scr4[:, :, 0:1],
                    na_all[:, g * G:(g + 1) * G])
            sq_na = dstat.tile([P, NCHUNK], f32, tag="sqna")
            nc.scalar.activation(sq_na[:], na_all[:], AF.Sqrt)
            nc.vector.reciprocal(rn_all[:], sq_na[:])
            for g in range(NGROUP):
                o_sb4 = dmisc.tile([P, G, D], f32, tag="osb")
                nc.vector.tensor_mul(
                    o_sb4[:], o_keep[g][:],
                    _bcast3(bass, rn_all[:, g * G:(g + 1) * G], D))
                nc.sync.dma_start(
                    out=out_d[g * G * P:(g + 1) * G * P, :].rearrange(
                        "(c p) d -> p c d", p=P),
                    in_=o_sb4[:])

    nc.compile()
    return nc


def _get_runner(cfg_key, consts):
    if cfg_key in _RUNNER_CACHE:
        return _RUNNER_CACHE[cfg_key]
    cfg = dict(mm_qk=consts["mm_qk"], mm_av=consts["mm_av"],
               mm_proj=consts["mm_proj"])
    nc = _build_program(cfg, consts)
    _RUNNER_CACHE[cfg_key] = nc
    return nc


# ---------------------------------------------------------------- host logic
def _host_fixup_rows(out, value, mask, Wv, bv, sv):
    """Exactly reproduce reference for rows with no allowed keys."""
    for b in range(B):
        cnt = np.cumsum(~mask[b])
        rows = np.where(cnt == 0)[0]
        if rows.size == 0:
            continue
        x = value[b].astype(np.float32) @ Wv.T.astype(np.float32) + bv
        time = 1.0 / (1.0 + np.exp(-x[:, :1])) * np.exp(sv) + 1.1
        xn = x[:, 1:]
        s = (time * time - 1.0) / np.sum(xn * xn, axis=-1, keepdims=True)
        vproj = np.concatenate([time, xn * np.sqrt(s)], axis=-1)
        ave = vproj.mean(axis=0)
        lor = -ave[0] ** 2 + np.sum(ave[1:] ** 2)
        denom = np.sqrt(max(abs(lor), 1e-8))
        out[b, rows] = (ave / denom).astype(np.float32)


def kernel(query, key, value, mask, Wq, bq, sq, Wk, bk, sk, Wv, bv, sv,
           attn_scale, attn_bias):
    from concourse.bass_utils import run_bass_kernel_spmd

    query = np.asarray(query, dtype=np.float32)
    key = np.asarray(key, dtype=np.float32)
    value = np.asarray(value, dtype=np.float32)
    mask = np.asarray(mask).astype(bool)
    Wq, Wk, Wv = (np.asarray(w, dtype=np.float32) for w in (Wq, Wk, Wv))
    bq, bk, bv = (np.asarray(b, dtype=np.float32).reshape(-1)
                  for b in (bq, bk, bv))

    has_bias = bool(np.any(bq) or np.any(bk) or np.any(bv))
    consts = dict(
        es_q=float(np.exp(np.float32(sq))),
        es_k=float(np.exp(np.float32(sk))),
        es_v=float(np.exp(np.float32(sv))),
        c1=float(2.0 / np.asarray(attn_scale, dtype=np.float32).reshape(-1)[0]),
        has_bias=has_bias,
        mm_qk=os.environ.get("LK_MM_QK", "f32r"),
        mm_av=os.environ.get("LK_MM_AV", "f32"),
        mm_proj=os.environ.get("LK_MM_PROJ", "f32"),
    )
    cfg_key = tuple(sorted(consts.items()))
    nc = _get_runner(cfg_key, consts)

    pad01 = (~mask).astype(np.float32)
    wt = {"q": np.ascontiguousarray(Wq.T), "k": np.ascontiguousarray(Wk.T),
          "v": np.ascontiguousarray(Wv.T)}
    in_maps = []
    for b in range(B):
        m = {
            "q": np.ascontiguousarray(query[b].T),
            "k": np.ascontiguousarray(key[b].T),
            "v": np.ascontiguousarray(value[b].T),
            "pad": pad01[b],
            "wqt": wt["q"], "wkt": wt["k"], "wvt": wt["v"],
        }
        if has_bias:
            m["bq"] = bq.reshape(1, D)
            m["bk"] = bk.reshape(1, D)
            m["bv"] = bv.reshape(1, D)
        in_maps.append(m)

    res = run_bass_kernel_spmd(nc, in_maps, core_ids=list(range(B)))
    out = np.stack([res.results[b]["out"] for b in range(B)], axis=0)
    _host_fixup_rows(out, value, mask, Wv, bv, float(np.float32(sv)))
    return out



# revision 16
# speedup vs baseline: 1.0951x; 1.0951x over previous
"""Trainium2 Bass kernel for LorentzSelfAttention (B=8, L=2048, D=128, 1 head).

Sharding: data-parallel over batch — core b handles batch element b.

Per-core algorithm (one NeuronCore, L=2048, D=128, 16 row-chunks of 128):
  Inputs arrive HOST-TRANSPOSED: qT/kT/vT [D, L] so no on-device input
  transposes are needed (the contraction dim must sit on partitions).
  Phase B (projections), grouped 4 chunks at a time:
      linT chunk c: matmul(lhsT=xT[:, c*128:(c+1)*128], rhs=W^T) -> natural
      [l, dout] PSUM. Row-wise Lorentz stats (sigmoid / sum-of-squares) are
      batched into [128, 48] stat tiles so ONE ACT Sqrt serves all 48 chunks
      (Sigmoid, Sqrt, Exp live in different ACT table sets — interleaving
      them costs a ~1.3us table load per switch; Square/Copy are in every
      set and are free). q/k chunks are PE-transposed into qT/kT [d, l];
      v stays natural with padded rows zeroed.
  Phase C (attention): scores computed TRANSPOSED, S_T[j, i] = <k_j, q_i>_L
      (q time row negated), only causal columns i >= j*128, float32r matmuls.
      exp() applied without max-subtract / row-sum normalization: the final
      Lorentz mid-point normalization out = ave/sqrt(|<ave,ave>_L|) is
      scale-invariant per row, so all softmax constants cancel. Pad masking
      is folded into v (zeroed rows); causal diag-block masking is a 0/1
      upper-tri multiply. AV accumulates transposed: outT[d, i] += v_j.T @
      expT_j (PSUM, 16 steps, float32r).
  Phase D: PE-transpose outT back to natural (grouped 4 chunks / PSUM bank),
      batched row-wise Lorentz normalize (one Sqrt), DMA out.

Rows whose allowed (causal & non-pad) key set is empty produce softmax over
an all -inf row in the reference (== uniform over ALL 2048 keys). Those rows
(a ~0-2 row prefix per batch, only when the batch's first keys are padded)
are fixed up exactly on host.
"""

import os

import numpy as np

B, L, D = 8, 2048, 128
P = 128
NCHUNK = L // P   # 16
G = 4             # chunks per group
NGROUP = NCHUNK // G  # 4

_RUNNER_CACHE: dict = {}


def _bcast3(bass, ap2, inner):
    """[P, n] AP -> [P, n, inner] broadcast view (step-0 innermost)."""
    return bass.AP(tensor=ap2.tensor, offset=ap2.offset,
                   ap=[ap2.ap[0], ap2.ap[1], [0, inner]])


# ---------------------------------------------------------------- device code
def _build_program(cfg, consts):
    from contextlib import ExitStack

    import concourse.bacc as bacc
    import concourse.bass as bass
    import concourse.mybir as mybir
    import concourse.tile as tile
    from concourse import masks

    f32 = mybir.dt.float32
    bf16 = mybir.dt.bfloat16
    AF = mybir.ActivationFunctionType
    OP = mybir.AluOpType

    es = {"q": consts["es_q"], "k": consts["es_k"], "v": consts["es_v"]}
    c1 = consts["c1"]
    has_bias = consts["has_bias"]

    f32r = mybir.dt.float32r
    qk_dt = bf16   # q/k path: bf16 matmuls (1 cyc/row)
    av_dt = f32r   # v/exp path: f32r (1 cyc/row, near-f32 precision for na)

    nc = bacc.Bacc("TRN2", target_bir_lowering=False, debug=False)

    xT_d = {}
    for nm in ("q", "k", "v"):
        xT_d[nm] = nc.dram_tensor(nm, [D, L], bf16, kind="ExternalInput").ap()
    pad_d = nc.dram_tensor("pad", [L], f32, kind="ExternalInput").ap()
    wt_d = {nm: nc.dram_tensor(f"w{nm}t", [D, D], bf16, kind="ExternalInput").ap()
            for nm in ("q", "k", "v")}
    bias_d = {}
    if has_bias:
        for nm in ("q", "k", "v"):
            bias_d[nm] = nc.dram_tensor(f"b{nm}", [1, D], f32,
                                        kind="ExternalInput").ap()
    out_d = nc.dram_tensor("out", [L, D], f32, kind="ExternalOutput").ap()

    TENSORS = ("q", "k", "v")

    with tile.TileContext(nc) as tc, ExitStack() as octx:
        cpool = octx.enter_context(tc.tile_pool(name="consts", bufs=1))
        ident = cpool.tile([P, P], bf16)
        masks.make_identity(nc, ident[:])
        identf = cpool.tile([P, P], f32)
        masks.make_identity(nc, identf[:])
        ut01 = cpool.tile([P, P], f32)
        masks.make_upper_triangular(nc, ut01[:], val=1.0, diag=True)
        w_sb = {}
        for nm in TENSORS:
            w_sb[nm] = cpool.tile([P, D], bf16, name=f"w_{nm}", tag=f"w_{nm}")
            nc.sync.dma_start(out=w_sb[nm][:], in_=wt_d[nm][:, :])
        pad_sb = cpool.tile([P, NCHUNK], f32)
        nc.sync.dma_start(out=pad_sb[:], in_=pad_d.rearrange("(c p) -> p c", p=P))
        bias_sb = {}
        if has_bias:
            for nm in TENSORS:
                bt = cpool.tile([P, D], f32)
                bd = bias_d[nm]
                nc.sync.dma_start(out=bt[:], in_=bass.AP(
                    tensor=bd.tensor, offset=bd.offset, ap=[[0, P], bd.ap[1]]))
                bias_sb[nm] = bt

        # persistent activations
        qT_sb = cpool.tile([P, L], qk_dt)      # [d, l], time row negated
        kT_sb = cpool.tile([P, L], qk_dt)
        v_sb = cpool.tile([P, NCHUNK, D], av_dt)  # [l%128, chunk, d], pad-zeroed
        qk_nat = cpool.tile([P, 2, NCHUNK, D], bf16)  # q/k chunks, natural
        outT_sb = cpool.tile([P, L], f32)

        # batched per-row stats: col t*16+c is chunk c of tensor t
        time_all = cpool.tile([P, 3 * NCHUNK], f32)
        ss_all = cpool.tile([P, 3 * NCHUNK], f32)
        sqs_all = cpool.tile([P, 3 * NCHUNK], f32)

        # ---------------- Phase B: projections ----------------
        with ExitStack() as ctxB:
            xin = ctxB.enter_context(tc.tile_pool(name="xin", bufs=3))
            ps_l = ctxB.enter_context(tc.tile_pool(name="ps_l", bufs=3, space="PSUM"))
            ps_q = ctxB.enter_context(tc.tile_pool(name="ps_q", bufs=2, space="PSUM"))
            misc = ctxB.enter_context(tc.tile_pool(name="misc", bufs=3))
            stat = ctxB.enter_context(tc.tile_pool(name="stat", bufs=4))

            def dest4(nm, g):
                if nm == "v":
                    return v_sb[:, g * G:(g + 1) * G, :]
                ti = TENSORS.index(nm)
                return qk_nat[:, ti, g * G:(g + 1) * G, :]

            # pass 1: matmuls + sigmoid/square stats (ACT stays in the
            # sigmoid table set: Sigmoid+Square only), narrow -> SBUF dest
            for g in range(NGROUP):
                for nm in TENSORS:
                    ti = TENSORS.index(nm)
                    sb = ti * NCHUNK + g * G   # stats col base
                    xT4 = xin.tile([P, G * P], bf16, tag=f"x{nm}")
                    nc.sync.dma_start(
                        out=xT4[:], in_=xT_d[nm][:, g * G * P:(g + 1) * G * P])
                    lin4 = ps_l.tile([P, G * D], f32, tag="lin")
                    for c in range(G):
                        nc.tensor.matmul(
                            lin4[:, c * D:(c + 1) * D],
                            xT4[:, c * P:(c + 1) * P],
                            w_sb[nm][:], start=True, stop=True)
                    if has_bias:
                        lin_sb4 = misc.tile([P, G * D], f32, tag="linb")
                        nc.vector.tensor_add(
                            lin_sb4[:], lin4[:],
                            bass.AP(tensor=bias_sb[nm].tensor,
                                    offset=bias_sb[nm][:].offset,
                                    ap=[bias_sb[nm][:].ap[0], [0, G], [1, D]]))
                        src4 = lin_sb4[:].rearrange("p (c d) -> p c d", d=D)
                    else:
                        src4 = lin4[:].rearrange("p (c d) -> p c d", d=D)
                    # sigmoid of column 0 -> sig (into time_all slot, raw)
                    nc.scalar.activation(
                        time_all[:, sb:sb + G], src4[:, :, 0:1], AF.Sigmoid)
                    # sum of squares of narrow part
                    sq4 = misc.tile([P, G, D - 1], bf16, tag="sq4")
                    nc.scalar.activation(sq4[:], src4[:, :, 1:D], AF.Square)
                    nc.vector.tensor_reduce(
                        ss_all[:, sb:sb + G], sq4[:], mybir.AxisListType.X,
                        OP.add)
                    # park unscaled narrow in its SBUF destination (frees PSUM)
                    nc.vector.tensor_copy(dest4(nm, g)[:, :, 1:D],
                                          src4[:, :, 1:D])

            # pass 2: batched scalar math over all 48 chunks
            #   time = sig*exp(s) + 1.1  (per-tensor scale)
            for nm in TENSORS:
                ti = TENSORS.index(nm)
                sb = ti * NCHUNK
                nc.vector.tensor_scalar(
                    out=time_all[:, sb:sb + NCHUNK],
                    in0=time_all[:, sb:sb + NCHUNK],
                    scalar1=es[nm], scalar2=1.1, op0=OP.mult, op1=OP.add)
            inv_ss = stat.tile([P, 3 * NCHUNK], f32, tag="iss")
            nc.vector.reciprocal(inv_ss[:], ss_all[:])
            t2m1 = stat.tile([P, 3 * NCHUNK], f32, tag="t2m1")
            nc.vector.tensor_mul(t2m1[:], time_all[:], time_all[:])
            nc.vector.tensor_scalar_add(out=t2m1[:], in0=t2m1[:], scalar1=-1.0)
            sval = stat.tile([P, 3 * NCHUNK], f32, tag="sval")
            nc.vector.tensor_mul(sval[:], t2m1[:], inv_ss[:])
            nc.scalar.activation(sqs_all[:], sval[:], AF.Sqrt)
            # fold pad zeroing into v scales (time + narrow) — AFTER the sqrt
            vb = 2 * NCHUNK
            nc.vector.tensor_mul(
                sqs_all[:, vb:vb + NCHUNK], sqs_all[:, vb:vb + NCHUNK], pad_sb[:])
            nc.vector.tensor_mul(
                time_all[:, vb:vb + NCHUNK], time_all[:, vb:vb + NCHUNK],
                pad_sb[:])

            # pass 3: finish chunks in place, transpose q/k into qT/kT
            for g in range(NGROUP):
                for nm in TENSORS:
                    ti = TENSORS.index(nm)
                    sb = ti * NCHUNK + g * G
                    ch4 = dest4(nm, g)
                    tsign = -1.0 if nm == "q" else 1.0
                    # signed time into col 0
                    nc.vector.tensor_scalar(
                        out=ch4[:, :, 0:1], in0=time_all[:, sb:sb + G],
                        scalar1=tsign, scalar2=0.0, op0=OP.mult, op1=OP.add)
                    # narrow scaled by sqrt(s) in place (per-chunk broadcast)
                    nc.vector.tensor_mul(
                        ch4[:, :, 1:D], ch4[:, :, 1:D],
                        _bcast3(bass, sqs_all[:, sb:sb + G], D - 1))
                    if nm != "v":
                        qkT4 = ps_q.tile([P, G * P], bf16, tag="qkT")
                        for c in range(G):
                            nc.tensor.transpose(
                                qkT4[:, c * P:(c + 1) * P], ch4[:, c, :],
                                ident[:])
                        dst = qT_sb if nm == "q" else kT_sb
                        nc.vector.tensor_copy(
                            dst[:, g * G * P:(g + 1) * G * P], qkT4[:])

        # ---------------- Phase C: attention ----------------
        with ExitStack() as ctxC:
            ps_s = ctxC.enter_context(tc.tile_pool(name="ps_s", bufs=2, space="PSUM"))
            ps_o = ctxC.enter_context(tc.tile_pool(name="ps_o", bufs=1, space="PSUM"))
            sb_e = ctxC.enter_context(tc.tile_pool(name="sb_e", bufs=2))
            outT_ps = ps_o.tile([P, L], f32)

            for j in range(NCHUNK):
                ncols = (NCHUNK - j) * P
                base = j * P
                expT = sb_e.tile([P, L], av_dt, tag="expT")  # col0 == global i=base
                kblk = kT_sb[:, base:base + P]
                ofs = 0
                while ofs < ncols:   # scores + exp in <=1024-col slabs
                    sw = min(1024, ncols - ofs)
                    s_ps = ps_s.tile([P, 1024], f32, tag="s")
                    mofs = 0
                    while mofs < sw:  # matmul N<=512 per PSUM bank
                        w = min(512, sw - mofs)
                        nc.tensor.matmul(
                            s_ps[:, mofs:mofs + w], kblk,
                            qT_sb[:, base + ofs + mofs:base + ofs + mofs + w],
                            start=True, stop=True)
                        mofs += w
                    nc.scalar.activation(
                        expT[:, ofs:ofs + sw], s_ps[:, :sw], AF.Exp, scale=c1)
                    ofs += sw
                # causal mask inside the diagonal block
                nc.vector.tensor_mul(expT[:, 0:P], expT[:, 0:P], ut01[:])
                # outT[d, i] += v_j.T @ expT_j  (bank-aligned psum chunks)
                col = base
                while col < L:
                    bank_end = min(L, (col // 512 + 1) * 512)
                    kbank = bank_end // 512 - 1
                    last_j = 4 * kbank + 3
                    nc.tensor.matmul(
                        outT_ps[:, col:bank_end],
                        v_sb[:, j, :],
                        expT[:, col - base:bank_end - base],
                        start=(j == 0), stop=(j == last_j))
                    col = bank_end

            nc.vector.tensor_copy(outT_sb[:], outT_ps[:])

        # ---------------- Phase D: normalize + store ----------------
        with ExitStack() as ctxD:
            ps_d = ctxD.enter_context(tc.tile_pool(name="ps_d", bufs=4, space="PSUM"))
            dmisc = ctxD.enter_context(tc.tile_pool(name="dmisc", bufs=3))
            dstat = ctxD.enter_context(tc.tile_pool(name="dstat", bufs=2))
            na_all = dstat.tile([P, NCHUNK], f32, tag="na")
            rn_all = dstat.tile([P, NCHUNK], f32, tag="rn")
            o_keep = {}
            for g in range(NGROUP):
                o_ps4 = ps_d.tile([P, G, D], f32, tag="o")
                o_keep[g] = o_ps4
                for c in range(G):
                    nc.tensor.transpose(
                        o_ps4[:, c, :],
                        outT_sb[:, (g * G + c) * P:(g * G + c + 1) * P],
                        identf[:])
                scr4 = dmisc.tile([P, G, D], f32, tag="scr")
                nc.scalar.activation(scr4[:], o_ps4[:], AF.Square)
                # ssum (t^2 + |n|^2) then na = -lor = 2*t^2 - ssum
                nc.vector.tensor_reduce(
                    na_all[:, g * G:(g + 1) * G], scr4[:],
                    mybir.AxisListType.X, OP.add)
                nc.vector.tensor_scalar(
                    out=scr4[:, :, 0:1], in0=scr4[:, :, 0:1], scalar1=2.0,
                    scalar2=0.0, op0=OP.mult, op1=OP.add)
                nc.vector.tensor_sub(
                    na_all[:, g * G:(g + 1) * G], scr4[:, :, 0:1],
                    na_all[:, g * G:(g + 1) * G])
            sq_na = dstat.tile([P, NCHUNK], f32, tag="sqna")
            nc.scalar.activation(sq_na[:], na_all[:], AF.Sqrt)
            nc.vector.reciprocal(rn_all[:], sq_na[:])
            for g in range(NGROUP):
                o_sb4 = dmisc.tile([P, G, D], f32, tag="osb")
                nc.vector.tensor_mul(
                    o_sb4[:], o_keep[g][:],
                    _bcast3(bass, rn_all[:, g * G:(g + 1) * G], D))
                nc.sync.dma_start(
                    out=out_d[g * G * P:(g + 1) * G * P, :].rearrange(
                        "(c p) d -> p c d", p=P),
                    in_=o_sb4[:])

    nc.compile()
    return nc


def _get_runner(cfg_key, consts):
    if cfg_key in _RUNNER_CACHE:
        return _RUNNER_CACHE[cfg_key]
    cfg = dict(mm_qk=consts["mm_qk"], mm_av=consts["mm_av"],
               mm_proj=consts["mm_proj"])
    nc = _build_program(cfg, consts)
    _RUNNER_CACHE[cfg_key] = nc
    return nc


# ---------------------------------------------------------------- host logic
def _host_fixup_rows(out, value, mask, Wv, bv, sv):
    """Exactly reproduce reference for rows with no allowed keys."""
    for b in range(B):
        cnt = np.cumsum(~mask[b])
        rows = np.where(cnt == 0)[0]
        if rows.size == 0:
            continue
        x = value[b].astype(np.float32) @ Wv.T.astype(np.float32) + bv
        time = 1.0 / (1.0 + np.exp(-x[:, :1])) * np.exp(sv) + 1.1
        xn = x[:, 1:]
        s = (time * time - 1.0) / np.sum(xn * xn, axis=-1, keepdims=True)
        vproj = np.concatenate([time, xn * np.sqrt(s)], axis=-1)
        ave = vproj.mean(axis=0)
        lor = -ave[0] ** 2 + np.sum(ave[1:] ** 2)
        denom = np.sqrt(max(abs(lor), 1e-8))
        out[b, rows] = (ave / denom).astype(np.float32)


def kernel(query, key, value, mask, Wq, bq, sq, Wk, bk, sk, Wv, bv, sv,
           attn_scale, attn_bias):
    from ml_dtypes import bfloat16
    from concourse.bass_utils import run_bass_kernel_spmd

    query = np.asarray(query, dtype=np.float32)
    key = np.asarray(key, dtype=np.float32)
    value = np.asarray(value, dtype=np.float32)
    mask = np.asarray(mask).astype(bool)
    Wq, Wk, Wv = (np.asarray(w, dtype=np.float32) for w in (Wq, Wk, Wv))
    bq, bk, bv = (np.asarray(b, dtype=np.float32).reshape(-1)
                  for b in (bq, bk, bv))

    has_bias = bool(np.any(bq) or np.any(bk) or np.any(bv))
    consts = dict(
        es_q=float(np.exp(np.float32(sq))),
        es_k=float(np.exp(np.float32(sk))),
        es_v=float(np.exp(np.float32(sv))),
        c1=float(2.0 / np.asarray(attn_scale, dtype=np.float32).reshape(-1)[0]),
        has_bias=has_bias,
        mm_qk=os.environ.get("LK_MM_QK", "f32r"),
        mm_av=os.environ.get("LK_MM_AV", "f32"),
        mm_proj=os.environ.get("LK_MM_PROJ", "f32"),
    )
    cfg_key = tuple(sorted(consts.items()))
    nc = _get_runner(cfg_key, consts)

    pad01 = (~mask).astype(np.float32)
    wt = {"q": np.ascontiguousarray(Wq.T).astype(bfloat16),
          "k": np.ascontiguousarray(Wk.T).astype(bfloat16),
          "v": np.ascontiguousarray(Wv.T).astype(bfloat16)}
    in_maps = []
    for b in range(B):
        m = {
            "q": np.ascontiguousarray(query[b].T).astype(bfloat16),
            "k": np.ascontiguousarray(key[b].T).astype(bfloat16),
            "v": np.ascontiguousarray(value[b].T).astype(bfloat16),
            "pad": pad01[b],
            "wqt": wt["q"], "wkt": wt["k"], "wvt": wt["v"],
        }
        if has_bias:
            m["bq"] = bq.reshape(1, D)
            m["bk"] = bk.reshape(1, D)
            m["bv"] = bv.reshape(1, D)
        in_maps.append(m)

    res = run_bass_kernel_spmd(nc, in_maps, core_ids=list(range(B)))
    out = np.stack([res.results[b]["out"] for b in range(B)], axis=0)
    _host_fixup_rows(out, value, mask, Wv, bv, float(np.float32(sv)))
    return out



# revision 30
# speedup vs baseline: 1.1277x; 1.0297x over previous
"""Trainium2 Bass kernel for LorentzSelfAttention (B=8, L=2048, D=128, 1 head).

Sharding: data-parallel over batch — core b handles batch element b.

Per-core pipeline (L=2048, D=128, 16 row-chunks of 128, 4 groups of 4):
  Inputs arrive HOST-TRANSPOSED and bf16: xT [D, L] per tensor, loaded with
  ONE full-tensor DMA each on separate DMA queues (sync/scalar/gpsimd) so
  transfers overlap the framework preamble and each other. Weights wT for
  q/k/v plus the pad row are packed into a single [D, 3D+16] bf16 DMA.

  ONE ACT table (exp_and_others) for the whole kernel: sigmoid is computed
  as 0.5*tanh(x/2)+0.5 (tanh lives in the exp table), sqrt/rsqrt via DVE
  bit-trick + Newton (reciprocal), exp for attention. No mid-kernel
  ACT_TABLE_LOADs and no batched-stats sync point.

  Phase B (per group g, software-pipelined):
    12 bf16 matmuls (x-chunk stationary) -> PSUM [l, dout] f32; tanh of
    col 0 and Square+reduce of narrow cols read PSUM directly; per-group
    stats ([P, 12]) -> time / sqrt(s) via DVE Newton; narrow scaled
    PSUM->SBUF in one op (q/k: bf16, v: f32r with pad folded in); q/k
    chunks PE-transposed (bf16, 1 cyc/row) into qT/kT. Transposes of
    group g are emitted after group g+1's matmuls so the PE never waits
    on the stats chain.

  Phase C: scores transposed S_T[j, i] = <k_j, q_i>_L, bf16 matmuls in
    512-col slabs, exp (unnormalized — final Lorentz normalization is
    scale-invariant so softmax constants cancel) -> f32r expT; causal
    diag-block mask multiply on GpSimd; AV accumulates transposed in a
    4-bank PSUM tile outT_ps[d, i] via f32r matmuls (1 cyc/row).

  Phase D is folded INTO Phase C per PSUM bank: bank b of outT completes
    at j=4b+3, so its copy-out (GpSimd), PE transposes back to natural,
    Lorentz-norm stats (Square on GpSimd, reduce + rsqrt Newton on DVE)
    and the per-bank output DMA all overlap later j iterations.

Rows with an empty allowed key set (softmax over all -inf) are fixed up
exactly on host (a ~0-2 row prefix per batch).
"""

import numpy as np

B, L, D = 8, 2048, 128
P = 128
NCHUNK = L // P   # 16
G = 4             # chunks per group
NGROUP = NCHUNK // G  # 4
NBANK = 4         # 512-col PSUM banks of outT

_RUNNER_CACHE: dict = {}

MAGIC_SQRT = 0x1FBD1DF5


def _bcast3(bass, ap2, inner):
    """[P, n] AP -> [P, n, inner] broadcast view (step-0 innermost)."""
    return bass.AP(tensor=ap2.tensor, offset=ap2.offset,
                   ap=[ap2.ap[0], ap2.ap[1], [0, inner]])


# ---------------------------------------------------------------- device code
def _build_program(consts):
    from contextlib import ExitStack

    import concourse.bacc as bacc
    import concourse.bass as bass
    import concourse.mybir as mybir
    import concourse.tile as tile
    from concourse import masks

    f32 = mybir.dt.float32
    f32r = mybir.dt.float32r
    bf16 = mybir.dt.bfloat16
    i32 = mybir.dt.int32
    AF = mybir.ActivationFunctionType
    OP = mybir.AluOpType

    es = {"q": consts["es_q"], "k": consts["es_k"], "v": consts["es_v"]}
    c1 = consts["c1"]
    has_bias = consts["has_bias"]

    nc = bacc.Bacc("TRN2", target_bir_lowering=False, debug=False)

    xT_d = {}
    for nm in ("q", "k", "v"):
        xT_d[nm] = nc.dram_tensor(nm, [D, L], bf16, kind="ExternalInput").ap()
    # packed: wqT | wkT | wvT | pad(as [P, NCHUNK])
    wp_d = nc.dram_tensor("wpack", [D, 3 * D + NCHUNK], bf16,
                          kind="ExternalInput").ap()
    bias_d = {}
    if has_bias:
        for nm in ("q", "k", "v"):
            bias_d[nm] = nc.dram_tensor(f"b{nm}", [1, D], f32,
                                        kind="ExternalInput").ap()
    out_d = nc.dram_tensor("out", [L, D], f32, kind="ExternalOutput").ap()
    debug = consts.get("debug", False)
    if debug:
        dbg_d = {nm: nc.dram_tensor(f"dbg_{nm}", [D, L], f32,
                                    kind="ExternalOutput").ap()
                 for nm in ("qT", "kT", "outT")}
        dbgv_d = nc.dram_tensor("dbg_v", [P, NCHUNK, D], f32,
                                kind="ExternalOutput").ap()

    import os as _os
    TENSORS = ("q", "k", "v")
    _dmaq = _os.environ.get("LK_DMAQ", "multi")
    if _dmaq == "sync":
        DMAQ = {"q": nc.sync, "k": nc.sync, "v": nc.sync}
        WPQ = nc.sync
    elif _dmaq == "scalar":
        DMAQ = {"q": nc.sync, "k": nc.scalar, "v": nc.scalar}
        WPQ = nc.scalar
    else:
        DMAQ = {"q": nc.sync, "k": nc.scalar, "v": nc.gpsimd}
        WPQ = nc.scalar

    with tile.TileContext(nc) as tc, ExitStack() as octx:
        cpool = octx.enter_context(tc.tile_pool(name="consts", bufs=1))

        # ---- inputs first: big DMAs on separate queues overlap preamble
        wpack = cpool.tile([P, 3 * D + NCHUNK], bf16)
        WPQ.dma_start(out=wpack[:], in_=wp_d[:, :])
        xsb = {}
        for nm in TENSORS:
            xsb[nm] = cpool.tile([P, L], bf16, name=f"x_{nm}", tag=f"x_{nm}")
            DMAQ[nm].dma_start(out=xsb[nm][:], in_=xT_d[nm][:, :])
        w_sb = {nm: wpack[:, ti * D:(ti + 1) * D]
                for ti, nm in enumerate(TENSORS)}
        pad_sb = wpack[:, 3 * D:3 * D + NCHUNK]   # 0/1 in bf16 (exact)
        bias_sb = {}
        if has_bias:
            for nm in TENSORS:
                bt = cpool.tile([P, D], f32, name=f"bias_{nm}",
                                tag=f"bias_{nm}")
                bd = bias_d[nm]
                nc.scalar.dma_start(out=bt[:], in_=bass.AP(
                    tensor=bd.tensor, offset=bd.offset, ap=[[0, P], bd.ap[1]]))
                bias_sb[nm] = bt

        ident = cpool.tile([P, P], bf16)
        masks.make_identity(nc, ident[:])
        identf = cpool.tile([P, P], f32)
        masks.make_identity(nc, identf[:])
        ut01 = cpool.tile([P, P], f32)
        masks.make_upper_triangular(nc, ut01[:], val=1.0, diag=True)

        # persistent activations
        qT_sb = cpool.tile([P, L], bf16)      # [d, l], time row negated
        kT_sb = cpool.tile([P, L], bf16)
        v_sb = cpool.tile([P, NCHUNK, D], f32r)  # [l%128, chunk, d], pad-zeroed

        # DVE sqrt: y = sqrt(x), 2 Newton iterations. ~7 tiny ops.
        def dve_sqrt(pool, x_ap, n, tag):
            y = pool.tile([P, n], f32, name=f"sq_{tag}", tag=f"sq_{tag}")
            nc.vector.tensor_scalar(out=y[:].bitcast(i32),
                                    in0=x_ap.bitcast(i32), scalar1=1,
                                    scalar2=None, op0=OP.arith_shift_right)
            nc.vector.tensor_scalar(out=y[:].bitcast(i32),
                                    in0=y[:].bitcast(i32), scalar1=MAGIC_SQRT,
                                    scalar2=None, op0=OP.add)
            for it in range(2):
                r = pool.tile([P, n], f32, name=f"r{it}_{tag}",
                              tag=f"r{it}_{tag}")
                nc.vector.reciprocal(r[:], y[:])
                nc.vector.scalar_tensor_tensor(
                    out=r[:], in0=x_ap, scalar=0.5, in1=r[:],
                    op0=OP.mult, op1=OP.mult)
                nc.vector.scalar_tensor_tensor(
                    out=y[:], in0=y[:], scalar=0.5, in1=r[:],
                    op0=OP.mult, op1=OP.add)
            return y

        # ---------------- Phase B: projections, per-group pipeline ----------
        with ExitStack() as ctxB:
            ps_l = ctxB.enter_context(
                tc.tile_pool(name="ps_l", bufs=2, space="PSUM"))
            ps_q = ctxB.enter_context(
                tc.tile_pool(name="ps_q", bufs=1, space="PSUM"))
            misc = ctxB.enter_context(tc.tile_pool(name="misc", bufs=2))
            stat = ctxB.enter_context(tc.tile_pool(name="stat", bufs=2))
            qknat = ctxB.enter_context(tc.tile_pool(name="qknat", bufs=2))

            lin_g = {}      # g -> {nm: psum tile}
            sqs_g = {}      # g -> [P, 12] sqrt(s), v cols pad-folded
            time_g = {}     # g -> [P, 12] time, v cols pad-folded
            nat_g = {}      # g -> {nm: scaled natural bf16 chunk (q/k only)}

            def emit_mm_stats(g):
                lin_g[g] = {}
                tg = stat.tile([P, 3 * G], f32, name=f"tg{g}", tag="tg")
                ssg = stat.tile([P, 3 * G], f32, name=f"ssg{g}", tag="ssg")
                for ti, nm in enumerate(TENSORS):
                    lin4 = ps_l.tile([P, G * D], f32, tag=f"lin_{nm}")
                    lin_g[g][nm] = lin4
                    for c in range(G):
                        nc.tensor.matmul(
                            lin4[:, c * D:(c + 1) * D],
                            xsb[nm][:, (g * G + c) * P:(g * G + c + 1) * P],
                            w_sb[nm], start=True, stop=True)
                    if has_bias:
                        nc.vector.tensor_add(
                            lin4[:], lin4[:],
                            bass.AP(tensor=bias_sb[nm].tensor,
                                    offset=bias_sb[nm][:].offset,
                                    ap=[bias_sb[nm][:].ap[0], [0, G], [1, D]]))
                    src4 = lin4[:].rearrange("p (c d) -> p c d", d=D)
                    # tanh(x/2) -> sigmoid pieces (exp-table resident)
                    nc.scalar.activation(
                        tg[:, ti * G:(ti + 1) * G], src4[:, :, 0:1],
                        AF.Tanh, scale=0.5)
                    sq4 = misc.tile([P, G, D - 1], bf16, name=f"sq4{nm}",
                                    tag=f"sq4_{nm}")
                    nc.scalar.activation(sq4[:], src4[:, :, 1:D], AF.Square)
                    nc.vector.tensor_reduce(
                        ssg[:, ti * G:(ti + 1) * G], sq4[:],
                        mybir.AxisListType.X, OP.add)
                # time = es*sigmoid + 1.1 = (es/2)*tanh + (es/2 + 1.1)
                for ti, nm in enumerate(TENSORS):
                    e2 = es[nm] * 0.5
                    nc.vector.tensor_scalar(
                        out=tg[:, ti * G:(ti + 1) * G],
                        in0=tg[:, ti * G:(ti + 1) * G],
                        scalar1=e2, scalar2=e2 + 1.1,
                        op0=OP.mult, op1=OP.add)
                # s = (time^2 - 1) / ssq ; sqs = sqrt(s)
                inv = stat.tile([P, 3 * G], f32, name=f"inv{g}", tag="inv")
                nc.vector.reciprocal(inv[:], ssg[:])
                sval = stat.tile([P, 3 * G], f32, name=f"sval{g}", tag="sval")
                nc.vector.tensor_mul(sval[:], tg[:], tg[:])
                nc.vector.tensor_scalar_add(out=sval[:], in0=sval[:],
                                            scalar1=-1.0)
                nc.vector.tensor_mul(sval[:], sval[:], inv[:])
                sqs = dve_sqrt(stat, sval[:], 3 * G, f"b{g}")
                # fold pad zeroing into v scale + time
                vb = 2 * G
                nc.vector.tensor_mul(sqs[:, vb:vb + G], sqs[:, vb:vb + G],
                                     pad_sb[:, g * G:(g + 1) * G])
                nc.vector.tensor_mul(tg[:, vb:vb + G], tg[:, vb:vb + G],
                                     pad_sb[:, g * G:(g + 1) * G])
                sqs_g[g] = sqs
                time_g[g] = tg
                # scale narrow PSUM -> SBUF dest; write time col
                nat_g[g] = {}
                for ti, nm in enumerate(TENSORS):
                    src4 = lin_g[g][nm][:].rearrange("p (c d) -> p c d", d=D)
                    if nm == "v":
                        dst = v_sb[:, g * G:(g + 1) * G, :]
                    else:
                        nat = qknat.tile([P, G, D], bf16, name=f"nat{nm}{g}",
                                         tag=f"nat_{nm}")
                        nat_g[g][nm] = nat
                        dst = nat[:]
                    tsign = -1.0 if nm == "q" else 1.0
                    nc.vector.tensor_scalar(
                        out=dst[:, :, 0:1], in0=tg[:, ti * G:(ti + 1) * G],
                        scalar1=tsign, scalar2=0.0, op0=OP.mult, op1=OP.add)
                    nc.vector.tensor_mul(
                        dst[:, :, 1:D], src4[:, :, 1:D],
                        _bcast3(bass, sqs[:, ti * G:(ti + 1) * G], D - 1))

            def emit_transposes(g):
                for nm in ("q", "k"):
                    qkT4 = ps_q.tile([P, G * P], bf16, tag=f"qkT_{nm}")
                    nat = nat_g[g][nm]
                    for c in range(G):
                        nc.tensor.transpose(
                            qkT4[:, c * P:(c + 1) * P], nat[:, c, :], ident[:])
                    dst = qT_sb if nm == "q" else kT_sb
                    nc.vector.tensor_copy(
                        dst[:, g * G * P:(g + 1) * G * P], qkT4[:])

            for g in range(NGROUP):
                emit_mm_stats(g)
                if g >= 1:
                    emit_transposes(g - 1)
            emit_transposes(NGROUP - 1)

        # ---------------- Phase C + per-bank Phase D ----------
        with ExitStack() as ctxC:
            ps_s = ctxC.enter_context(
                tc.tile_pool(name="ps_s", bufs=2, space="PSUM"))
            ps_o = ctxC.enter_context(
                tc.tile_pool(name="ps_o", bufs=1, space="PSUM"))
            ps_d = ctxC.enter_context(
                tc.tile_pool(name="ps_d", bufs=2, space="PSUM"))
            sb_e = ctxC.enter_context(tc.tile_pool(name="sb_e", bufs=2))
            dsb = ctxC.enter_context(tc.tile_pool(name="dsb", bufs=2))
            dstat = ctxC.enter_context(tc.tile_pool(name="dstat", bufs=2))

            outT_ps = ps_o.tile([P, L], f32)
            obank = {}   # b -> sbuf copy of finished bank
            otr = {}     # b -> natural transposed PSUM tile

            def emit_qk_exp(j):
                ncols = (NCHUNK - j) * P
                base = j * P
                expT = sb_e.tile([P, L], f32r, tag="expT")
                kblk = kT_sb[:, base:base + P]
                ofs = 0
                while ofs < ncols:
                    sw = min(512, ncols - ofs)
                    s_ps = ps_s.tile([P, 512], f32, tag="s")
                    nc.tensor.matmul(
                        s_ps[:, :sw], kblk,
                        qT_sb[:, base + ofs:base + ofs + sw],
                        start=True, stop=True)
                    nc.scalar.activation(
                        expT[:, ofs:ofs + sw], s_ps[:, :sw], AF.Exp, scale=c1)
                    ofs += sw
                # causal mask inside the diagonal block (gpsimd; writes f32r)
                if _os.environ.get("LK_UT01", "gpsimd") == "gpsimd":
                    nc.gpsimd.tensor_mul(expT[:, 0:P], expT[:, 0:P], ut01[:])
                else:
                    nc.vector.tensor_mul(expT[:, 0:P], expT[:, 0:P], ut01[:])
                return expT

            def emit_av(j, expT):
                base = j * P
                col = base
                while col < L:
                    bank_end = min(L, (col // 512 + 1) * 512)
                    kbank = bank_end // 512 - 1
                    last_j = 4 * kbank + 3
                    nc.tensor.matmul(
                        outT_ps[:, col:bank_end],
                        v_sb[:, j, :],
                        expT[:, col - base:bank_end - base],
                        start=(j == 0), stop=(j == last_j))
                    col = bank_end

            def emit_d_copy(b):
                ob = dsb.tile([P, NBANK, P], f32, name=f"ob{b}", tag="obank")
                obank[b] = ob
                nc.vector.tensor_copy(ob[:], outT_ps[:, 512 * b:512 * (b + 1)]
                                      .rearrange("p (c q) -> p c q", q=P))

            def emit_d_transposes(b):
                o_ps4 = ps_d.tile([P, NBANK, P], f32, tag="otr")
                otr[b] = o_ps4
                for c in range(NBANK):
                    nc.tensor.transpose(
                        o_ps4[:, c, :], obank[b][:, c, :], identf[:])

            def emit_d_stats(b):
                o_ps4 = otr[b]
                scr = dsb.tile([P, NBANK, P], f32, name=f"scr{b}", tag="scr")
                na = dstat.tile([P, NBANK], f32, name=f"na{b}", tag="na")
                nc.vector.tensor_copy(scr[:], o_ps4[:])
                nc.vector.tensor_mul(scr[:], scr[:], scr[:])
                nc.vector.tensor_reduce(na[:], scr[:], mybir.AxisListType.X,
                                        OP.add)
                tt = dstat.tile([P, NBANK], f32, name=f"tt{b}", tag="tt")
                nc.vector.tensor_scalar(
                    out=tt[:], in0=scr[:, :, 0:1], scalar1=2.0, scalar2=0.0,
                    op0=OP.mult, op1=OP.add)
                nc.vector.tensor_sub(na[:], tt[:], na[:])
                sqna = dve_sqrt(dstat, na[:], NBANK, f"d{b}")
                rn = dstat.tile([P, NBANK], f32, name=f"rn{b}", tag="rn")
                nc.vector.reciprocal(rn[:], sqna[:])
                osb = dsb.tile([P, NBANK, P], f32, name=f"osb{b}", tag="osb")
                nc.vector.tensor_mul(osb[:], o_ps4[:],
                                     _bcast3(bass, rn[:], P))
                nc.sync.dma_start(
                    out=out_d[b * 512:(b + 1) * 512, :].rearrange(
                        "(c p) d -> p c d", p=P),
                    in_=osb[:])

            overlap_d = _os.environ.get("LK_DOVERLAP", "1") == "1"
            for j in range(NCHUNK):
                if overlap_d and j > 0 and j % 4 == 0:
                    emit_d_copy(j // 4 - 1)
                expT = emit_qk_exp(j)
                if overlap_d and j > 0 and j % 4 == 0:
                    emit_d_transposes(j // 4 - 1)
                    emit_d_stats(j // 4 - 1)
                emit_av(j, expT)
            first_d = 3 if overlap_d else 0
            for bb in range(first_d, NBANK):
                emit_d_copy(bb)
                emit_d_transposes(bb)
                emit_d_stats(bb)

            if debug:
                dq = dsb.tile([P, L], f32, name="dq", tag="dbgq")
                nc.vector.tensor_copy(dq[:], qT_sb[:])
                nc.sync.dma_start(out=dbg_d["qT"][:, :], in_=dq[:])
                dk = dsb.tile([P, L], f32, name="dk", tag="dbgk")
                nc.vector.tensor_copy(dk[:], kT_sb[:])
                nc.sync.dma_start(out=dbg_d["kT"][:, :], in_=dk[:])
                dv = dsb.tile([P, NCHUNK, D], f32, name="dv", tag="dbgv")
                nc.vector.tensor_copy(dv[:], v_sb[:])
                nc.sync.dma_start(out=dbgv_d[:, :, :], in_=dv[:])
                do = dsb.tile([P, L], f32, name="do", tag="dbgo")
                for bb in range(NBANK):
                    nc.vector.tensor_copy(
                        do[:, 512 * bb:512 * (bb + 1)],
                        obank[bb][:].rearrange("p c q -> p (c q)"))
                nc.sync.dma_start(out=dbg_d["outT"][:, :], in_=do[:])

    nc.compile()
    return nc


def _get_runner(cfg_key, consts):
    if cfg_key in _RUNNER_CACHE:
        return _RUNNER_CACHE[cfg_key]
    nc = _build_program(consts)
    _RUNNER_CACHE[cfg_key] = nc
    return nc


# ---------------------------------------------------------------- host logic
def _host_fixup_rows(out, value, mask, Wv, bv, sv):
    """Exactly reproduce reference for rows with no allowed keys."""
    for b in range(B):
        cnt = np.cumsum(~mask[b])
        rows = np.where(cnt == 0)[0]
        if rows.size == 0:
            continue
        x = value[b].astype(np.float32) @ Wv.T.astype(np.float32) + bv
        time = 1.0 / (1.0 + np.exp(-x[:, :1])) * np.exp(sv) + 1.1
        xn = x[:, 1:]
        s = (time * time - 1.0) / np.sum(xn * xn, axis=-1, keepdims=True)
        vproj = np.concatenate([time, xn * np.sqrt(s)], axis=-1)
        ave = vproj.mean(axis=0)
        lor = -ave[0] ** 2 + np.sum(ave[1:] ** 2)
        denom = np.sqrt(max(abs(lor), 1e-8))
        out[b, rows] = (ave / denom).astype(np.float32)


def _pack_wpad(Wq, Wk, Wv, pad01):
    from ml_dtypes import bfloat16
    wp = np.zeros((D, 3 * D + NCHUNK), dtype=bfloat16)
    wp[:, 0:D] = Wq.T.astype(bfloat16)
    wp[:, D:2 * D] = Wk.T.astype(bfloat16)
    wp[:, 2 * D:3 * D] = Wv.T.astype(bfloat16)
    return wp


def kernel(query, key, value, mask, Wq, bq, sq, Wk, bk, sk, Wv, bv, sv,
           attn_scale, attn_bias):
    from ml_dtypes import bfloat16
    from concourse.bass_utils import run_bass_kernel_spmd

    query = np.asarray(query, dtype=np.float32)
    key = np.asarray(key, dtype=np.float32)
    value = np.asarray(value, dtype=np.float32)
    mask = np.asarray(mask).astype(bool)
    Wq, Wk, Wv = (np.asarray(w, dtype=np.float32) for w in (Wq, Wk, Wv))
    bq, bk, bv = (np.asarray(b, dtype=np.float32).reshape(-1)
                  for b in (bq, bk, bv))

    has_bias = bool(np.any(bq) or np.any(bk) or np.any(bv))
    consts = dict(
        es_q=float(np.exp(np.float32(sq))),
        es_k=float(np.exp(np.float32(sk))),
        es_v=float(np.exp(np.float32(sv))),
        c1=float(2.0 / np.asarray(attn_scale, dtype=np.float32).reshape(-1)[0]),
        has_bias=has_bias,
    )
    cfg_key = tuple(sorted(consts.items()))
    nc = _get_runner(cfg_key, consts)

    pad01 = (~mask).astype(np.float32)
    wbase = np.zeros((D, 3 * D + NCHUNK), dtype=bfloat16)
    wbase[:, 0:D] = Wq.T.astype(bfloat16)
    wbase[:, D:2 * D] = Wk.T.astype(bfloat16)
    wbase[:, 2 * D:3 * D] = Wv.T.astype(bfloat16)
    in_maps = []
    for b in range(B):
        wp = wbase.copy()
        wp[:, 3 * D:] = pad01[b].reshape(NCHUNK, P).T.astype(bfloat16)
        m = {
            "q": np.ascontiguousarray(query[b].T).astype(bfloat16),
            "k": np.ascontiguousarray(key[b].T).astype(bfloat16),
            "v": np.ascontiguousarray(value[b].T).astype(bfloat16),
            "wpack": wp,
        }
        if has_bias:
            m["bq"] = bq.reshape(1, D)
            m["bk"] = bk.reshape(1, D)
            m["bv"] = bv.reshape(1, D)
        in_maps.append(m)

    res = run_bass_kernel_spmd(nc, in_maps, core_ids=list(range(B)))
    out = np.stack([res.results[b]["out"] for b in range(B)], axis=0)
    _host_fixup_rows(out, value, mask, Wv, bv, float(np.float32(sv)))
    return out


# revision 37
# speedup vs baseline: 1.1765x; 1.0433x over previous
"""Trainium2 Bass kernel for LorentzSelfAttention (B=8, L=2048, D=128, 1 head).

Sharding: data-parallel over batch — core b handles batch element b.

Per-core pipeline (L=2048, D=128, 16 row-chunks of 128, 4 groups of 4):
  Inputs arrive HOST-TRANSPOSED and bf16: xT [D, L] per tensor, loaded with
  ONE full-tensor DMA each on separate DMA queues (sync/scalar/gpsimd) so
  transfers overlap the framework preamble and each other. Weights wT for
  q/k/v plus the pad row are packed into a single [D, 3D+16] bf16 DMA.

  ONE ACT table (exp_and_others) for the whole kernel: sigmoid is computed
  as 0.5*tanh(x/2)+0.5 (tanh lives in the exp table), sqrt/rsqrt via DVE
  bit-trick + Newton (reciprocal), exp for attention. No mid-kernel
  ACT_TABLE_LOADs and no batched-stats sync point.

  Phase B (per group g, software-pipelined):
    12 bf16 matmuls (x-chunk stationary) -> PSUM [l, dout] f32; tanh of
    col 0 and Square+reduce of narrow cols read PSUM directly; per-group
    stats ([P, 12]) -> time / sqrt(s) via DVE Newton; narrow scaled
    PSUM->SBUF in one op (q/k: bf16, v: f32r with pad folded in); q/k
    chunks PE-transposed (bf16, 1 cyc/row) into qT/kT. Transposes of
    group g are emitted after group g+1's matmuls so the PE never waits
    on the stats chain.

  Phase C: scores transposed S_T[j, i] = <k_j, q_i>_L, bf16 matmuls in
    512-col slabs, exp (unnormalized — final Lorentz normalization is
    scale-invariant so softmax constants cancel) -> f32r expT; causal
    diag-block mask multiply on GpSimd; AV accumulates transposed in a
    4-bank PSUM tile outT_ps[d, i] via f32r matmuls (1 cyc/row).

  Phase D is folded INTO Phase C per PSUM bank: bank b of outT completes
    at j=4b+3, so its copy-out (GpSimd), PE transposes back to natural,
    Lorentz-norm stats (Square on GpSimd, reduce + rsqrt Newton on DVE)
    and the per-bank output DMA all overlap later j iterations.

Rows with an empty allowed key set (softmax over all -inf) are fixed up
exactly on host (a ~0-2 row prefix per batch).
"""

import numpy as np

B, L, D = 8, 2048, 128
P = 128
NCHUNK = L // P   # 16
G = 4             # chunks per group
NGROUP = NCHUNK // G  # 4
NBANK = 4         # 512-col PSUM banks of outT

_RUNNER_CACHE: dict = {}

MAGIC_SQRT = 0x1FBD1DF5


def _bcast3(bass, ap2, inner):
    """[P, n] AP -> [P, n, inner] broadcast view (step-0 innermost)."""
    return bass.AP(tensor=ap2.tensor, offset=ap2.offset,
                   ap=[ap2.ap[0], ap2.ap[1], [0, inner]])


# ---------------------------------------------------------------- device code
def _build_program(consts):
    from contextlib import ExitStack

    import concourse.bacc as bacc
    import concourse.bass as bass
    import concourse.mybir as mybir
    import concourse.tile as tile
    from concourse import masks

    f32 = mybir.dt.float32
    f32r = mybir.dt.float32r
    bf16 = mybir.dt.bfloat16
    i32 = mybir.dt.int32
    AF = mybir.ActivationFunctionType
    OP = mybir.AluOpType

    es = {"q": consts["es_q"], "k": consts["es_k"], "v": consts["es_v"]}
    c1 = consts["c1"]
    has_bias = consts["has_bias"]

    nc = bacc.Bacc("TRN2", target_bir_lowering=False, debug=False)

    xT_d = {}
    for nm in ("q", "k", "v"):
        xT_d[nm] = nc.dram_tensor(nm, [D, L], bf16, kind="ExternalInput").ap()
    # packed: wqT | wkT | wvT | pad(as [P, NCHUNK])
    wp_d = nc.dram_tensor("wpack", [D, 3 * D + NCHUNK], bf16,
                          kind="ExternalInput").ap()
    bias_d = {}
    if has_bias:
        for nm in ("q", "k", "v"):
            bias_d[nm] = nc.dram_tensor(f"b{nm}", [1, D], f32,
                                        kind="ExternalInput").ap()
    out_d = nc.dram_tensor("out", [L, D], f32, kind="ExternalOutput").ap()
    debug = consts.get("debug", False)
    if debug:
        dbg_d = {nm: nc.dram_tensor(f"dbg_{nm}", [D, L], f32,
                                    kind="ExternalOutput").ap()
                 for nm in ("qT", "kT", "outT")}
        dbgv_d = nc.dram_tensor("dbg_v", [P, NCHUNK, D], f32,
                                kind="ExternalOutput").ap()

    import os as _os
    TENSORS = ("q", "k", "v")

    with tile.TileContext(nc) as tc, ExitStack() as octx:
        cpool = octx.enter_context(tc.tile_pool(name="consts", bufs=1))

        # ---- inputs first: big DMAs on separate queues overlap preamble
        wpack = cpool.tile([P, 3 * D + NCHUNK], bf16)
        nc.gpsimd.dma_start(out=wpack[:], in_=wp_d[:, :])
        xsb = {}
        for nm in TENSORS:
            xsb[nm] = cpool.tile([P, L], bf16, name=f"x_{nm}", tag=f"x_{nm}")
        # q split in halves so the first matmuls wait on less data
        nc.sync.dma_start(out=xsb["q"][:, :L // 2], in_=xT_d["q"][:, :L // 2])
        nc.sync.dma_start(out=xsb["q"][:, L // 2:], in_=xT_d["q"][:, L // 2:])
        nc.scalar.dma_start(out=xsb["k"][:], in_=xT_d["k"][:, :])
        nc.gpsimd.dma_start(out=xsb["v"][:], in_=xT_d["v"][:, :])
        w_sb = {nm: wpack[:, ti * D:(ti + 1) * D]
                for ti, nm in enumerate(TENSORS)}
        pad_sb = wpack[:, 3 * D:3 * D + NCHUNK]   # 0/1 in bf16 (exact)
        bias_sb = {}
        if has_bias:
            for nm in TENSORS:
                bt = cpool.tile([P, D], f32, name=f"bias_{nm}",
                                tag=f"bias_{nm}")
                bd = bias_d[nm]
                nc.scalar.dma_start(out=bt[:], in_=bass.AP(
                    tensor=bd.tensor, offset=bd.offset, ap=[[0, P], bd.ap[1]]))
                bias_sb[nm] = bt

        ident = cpool.tile([P, P], bf16)
        masks.make_identity(nc, ident[:])
        identf = cpool.tile([P, P], f32)
        masks.make_identity(nc, identf[:])
        ut01 = cpool.tile([P, P], f32)
        masks.make_upper_triangular(nc, ut01[:], val=1.0, diag=True)

        # persistent activations
        qT_sb = cpool.tile([P, L], bf16)      # [d, l], time row negated
        kT_sb = cpool.tile([P, L], bf16)
        v_sb = cpool.tile([P, NCHUNK, D], f32r)  # [l%128, chunk, d], pad-zeroed

        # DVE sqrt: y = sqrt(x) via bit-trick seed + Newton (reciprocal).
        # 1 iteration: ~1e-3 rel err; 2 iterations: ~5e-7.
        def dve_sqrt(pool, x_ap, n, tag, iters=2):
            y = pool.tile([P, n], f32, name=f"sq_{tag}", tag=f"sq_{tag}")
            nc.vector.tensor_scalar(out=y[:].bitcast(i32),
                                    in0=x_ap.bitcast(i32), scalar1=1,
                                    scalar2=None, op0=OP.arith_shift_right)
            nc.vector.tensor_scalar(out=y[:].bitcast(i32),
                                    in0=y[:].bitcast(i32), scalar1=MAGIC_SQRT,
                                    scalar2=None, op0=OP.add)
            for it in range(iters):
                r = pool.tile([P, n], f32, name=f"r{it}_{tag}",
                              tag=f"r{it}_{tag}")
                nc.vector.reciprocal(r[:], y[:])
                nc.vector.scalar_tensor_tensor(
                    out=r[:], in0=x_ap, scalar=0.5, in1=r[:],
                    op0=OP.mult, op1=OP.mult)
                nc.vector.scalar_tensor_tensor(
                    out=y[:], in0=y[:], scalar=0.5, in1=r[:],
                    op0=OP.mult, op1=OP.add)
            return y

        # ---------------- Phase B: projections, per-group pipeline ----------
        with ExitStack() as ctxB:
            ps_l = ctxB.enter_context(
                tc.tile_pool(name="ps_l", bufs=2, space="PSUM"))
            ps_q = ctxB.enter_context(
                tc.tile_pool(name="ps_q", bufs=1, space="PSUM"))
            misc = ctxB.enter_context(tc.tile_pool(name="misc", bufs=2))
            stat = ctxB.enter_context(tc.tile_pool(name="stat", bufs=2))
            qknat = ctxB.enter_context(tc.tile_pool(name="qknat", bufs=2))

            lin_g = {}      # g -> {nm: psum tile}
            sqs_g = {}      # g -> [P, 12] sqrt(s), v cols pad-folded
            time_g = {}     # g -> [P, 12] time, v cols pad-folded
            nat_g = {}      # g -> {nm: scaled natural bf16 chunk (q/k only)}

            def emit_mm_stats(g):
                lin_g[g] = {}
                tg = stat.tile([P, 3 * G], f32, name=f"tg{g}", tag="tg")
                ssg = stat.tile([P, 3 * G], f32, name=f"ssg{g}", tag="ssg")
                for ti, nm in enumerate(TENSORS):
                    lin4 = ps_l.tile([P, G * D], f32, tag=f"lin_{nm}")
                    lin_g[g][nm] = lin4
                    for c in range(G):
                        nc.tensor.matmul(
                            lin4[:, c * D:(c + 1) * D],
                            xsb[nm][:, (g * G + c) * P:(g * G + c + 1) * P],
                            w_sb[nm], start=True, stop=True)
                    if has_bias:
                        nc.vector.tensor_add(
                            lin4[:], lin4[:],
                            bass.AP(tensor=bias_sb[nm].tensor,
                                    offset=bias_sb[nm][:].offset,
                                    ap=[bias_sb[nm][:].ap[0], [0, G], [1, D]]))
                    src4 = lin4[:].rearrange("p (c d) -> p c d", d=D)
                    # tanh(x/2) -> sigmoid pieces (exp-table resident)
                    nc.scalar.activation(
                        tg[:, ti * G:(ti + 1) * G], src4[:, :, 0:1],
                        AF.Tanh, scale=0.5)
                    sq4 = misc.tile([P, G, D - 1], bf16, name=f"sq4{nm}",
                                    tag=f"sq4_{nm}")
                    nc.scalar.activation(sq4[:], src4[:, :, 1:D], AF.Square)
                    nc.vector.tensor_reduce(
                        ssg[:, ti * G:(ti + 1) * G], sq4[:],
                        mybir.AxisListType.X, OP.add)
                # time = es*sigmoid + 1.1 = (es/2)*tanh + (es/2 + 1.1)
                for ti, nm in enumerate(TENSORS):
                    e2 = es[nm] * 0.5
                    nc.vector.tensor_scalar(
                        out=tg[:, ti * G:(ti + 1) * G],
                        in0=tg[:, ti * G:(ti + 1) * G],
                        scalar1=e2, scalar2=e2 + 1.1,
                        op0=OP.mult, op1=OP.add)
                # s = (time^2 - 1) / ssq ; sqs = sqrt(s)
                inv = stat.tile([P, 3 * G], f32, name=f"inv{g}", tag="inv")
                nc.vector.reciprocal(inv[:], ssg[:])
                sval = stat.tile([P, 3 * G], f32, name=f"sval{g}", tag="sval")
                nc.vector.tensor_mul(sval[:], tg[:], tg[:])
                nc.vector.scalar_tensor_tensor(
                    out=sval[:], in0=sval[:], scalar=-1.0, in1=inv[:],
                    op0=OP.add, op1=OP.mult)
                sqs = dve_sqrt(stat, sval[:], 3 * G, f"b{g}", iters=1)
                # fold pad zeroing into v scale (time handled in its write)
                vb = 2 * G
                nc.vector.tensor_mul(sqs[:, vb:vb + G], sqs[:, vb:vb + G],
                                     pad_sb[:, g * G:(g + 1) * G])
                sqs_g[g] = sqs
                time_g[g] = tg
                # scale narrow PSUM -> SBUF dest; write time col
                nat_g[g] = {}
                for ti, nm in enumerate(TENSORS):
                    src4 = lin_g[g][nm][:].rearrange("p (c d) -> p c d", d=D)
                    if nm == "v":
                        dst = v_sb[:, g * G:(g + 1) * G, :]
                        # time col with pad fold in one op
                        nc.vector.tensor_mul(
                            dst[:, :, 0:1], tg[:, vb:vb + G],
                            pad_sb[:, g * G:(g + 1) * G])
                        # narrow scale on ACT: per-chunk Copy with
                        # per-partition scale (keeps DVE off the hot path)
                        for c in range(G):
                            nc.scalar.activation(
                                dst[:, c, 1:D], src4[:, c, 1:D], AF.Copy,
                                scale=sqs[:, vb + c:vb + c + 1])
                        continue
                    nat = qknat.tile([P, G, D], bf16, name=f"nat{nm}{g}",
                                     tag=f"nat_{nm}")
                    nat_g[g][nm] = nat
                    dst = nat[:]
                    tsign = -1.0 if nm == "q" else 1.0
                    nc.vector.tensor_scalar(
                        out=dst[:, :, 0:1], in0=tg[:, ti * G:(ti + 1) * G],
                        scalar1=tsign, scalar2=0.0, op0=OP.mult, op1=OP.add)
                    nc.vector.tensor_mul(
                        dst[:, :, 1:D], src4[:, :, 1:D],
                        _bcast3(bass, sqs[:, ti * G:(ti + 1) * G], D - 1))

            def emit_transposes(g):
                for nm in ("q", "k"):
                    qkT4 = ps_q.tile([P, G * P], bf16, tag=f"qkT_{nm}")
                    nat = nat_g[g][nm]
                    for c in range(G):
                        nc.tensor.transpose(
                            qkT4[:, c * P:(c + 1) * P], nat[:, c, :], ident[:])
                    dst = qT_sb if nm == "q" else kT_sb
                    nc.vector.tensor_copy(
                        dst[:, g * G * P:(g + 1) * G * P], qkT4[:])

            for g in range(NGROUP):
                emit_mm_stats(g)
                if g >= 1:
                    emit_transposes(g - 1)
            emit_transposes(NGROUP - 1)

        # ---------------- Phase C + per-bank Phase D ----------
        with ExitStack() as ctxC:
            ps_s = ctxC.enter_context(
                tc.tile_pool(name="ps_s", bufs=2, space="PSUM"))
            ps_o = ctxC.enter_context(
                tc.tile_pool(name="ps_o", bufs=1, space="PSUM"))
            ps_d = ctxC.enter_context(
                tc.tile_pool(name="ps_d", bufs=2, space="PSUM"))
            sb_e = ctxC.enter_context(tc.tile_pool(name="sb_e", bufs=2))
            dsb = ctxC.enter_context(tc.tile_pool(name="dsb", bufs=2))
            dstat = ctxC.enter_context(tc.tile_pool(name="dstat", bufs=2))

            outT_ps = ps_o.tile([P, L], f32)
            obank = {}   # b -> sbuf copy of finished bank
            otr = {}     # b -> natural transposed PSUM tile

            def emit_qk_exp(j):
                ncols = (NCHUNK - j) * P
                base = j * P
                expT = sb_e.tile([P, L], f32r, tag="expT")
                kblk = kT_sb[:, base:base + P]
                ofs = 0
                while ofs < ncols:
                    sw = min(512, ncols - ofs)
                    s_ps = ps_s.tile([P, 512], f32, tag="s")
                    nc.tensor.matmul(
                        s_ps[:, :sw], kblk,
                        qT_sb[:, base + ofs:base + ofs + sw],
                        start=True, stop=True)
                    nc.scalar.activation(
                        expT[:, ofs:ofs + sw], s_ps[:, :sw], AF.Exp, scale=c1)
                    ofs += sw
                # causal mask inside the diagonal block (gpsimd; writes f32r)
                if _os.environ.get("LK_UT01", "gpsimd") == "gpsimd":
                    nc.gpsimd.tensor_mul(expT[:, 0:P], expT[:, 0:P], ut01[:])
                else:
                    nc.vector.tensor_mul(expT[:, 0:P], expT[:, 0:P], ut01[:])
                return expT

            def emit_av(j, expT):
                base = j * P
                col = base
                while col < L:
                    bank_end = min(L, (col // 512 + 1) * 512)
                    kbank = bank_end // 512 - 1
                    last_j = 4 * kbank + 3
                    nc.tensor.matmul(
                        outT_ps[:, col:bank_end],
                        v_sb[:, j, :],
                        expT[:, col - base:bank_end - base],
                        start=(j == 0), stop=(j == last_j))
                    col = bank_end

            def emit_d_copy(b):
                ob = dsb.tile([P, NBANK, P], f32, name=f"ob{b}", tag="obank")
                obank[b] = ob
                nc.vector.tensor_copy(ob[:], outT_ps[:, 512 * b:512 * (b + 1)]
                                      .rearrange("p (c q) -> p c q", q=P))

            def emit_d_transposes(b):
                o_ps4 = ps_d.tile([P, NBANK, P], f32, tag="otr")
                otr[b] = o_ps4
                for c in range(NBANK):
                    nc.tensor.transpose(
                        o_ps4[:, c, :], obank[b][:, c, :], identf[:])

            def emit_d_stats(b):
                o_ps4 = otr[b]
                scr = dsb.tile([P, NBANK, P], f32, name=f"scr{b}", tag="scr")
                na = dstat.tile([P, NBANK], f32, name=f"na{b}", tag="na")
                nc.vector.tensor_copy(scr[:], o_ps4[:])
                nc.gpsimd.tensor_mul(scr[:], scr[:], scr[:])
                nc.vector.tensor_reduce(na[:], scr[:], mybir.AxisListType.X,
                                        OP.add)
                tt = dstat.tile([P, NBANK], f32, name=f"tt{b}", tag="tt")
                nc.vector.tensor_scalar(
                    out=tt[:], in0=scr[:, :, 0:1], scalar1=2.0, scalar2=0.0,
                    op0=OP.mult, op1=OP.add)
                nc.vector.tensor_sub(na[:], tt[:], na[:])
                sqna = dve_sqrt(dstat, na[:], NBANK, f"d{b}")
                rn = dstat.tile([P, NBANK], f32, name=f"rn{b}", tag="rn")
                nc.vector.reciprocal(rn[:], sqna[:])
                osb = dsb.tile([P, NBANK, P], f32, name=f"osb{b}", tag="osb")
                nc.vector.tensor_mul(osb[:], o_ps4[:],
                                     _bcast3(bass, rn[:], P))
                nc.sync.dma_start(
                    out=out_d[b * 512:(b + 1) * 512, :].rearrange(
                        "(c p) d -> p c d", p=P),
                    in_=osb[:])

            overlap_d = _os.environ.get("LK_DOVERLAP", "1") == "1"
            for j in range(NCHUNK):
                if overlap_d and j > 0 and j % 4 == 0:
                    emit_d_copy(j // 4 - 1)
                expT = emit_qk_exp(j)
                if overlap_d and j > 0 and j % 4 == 0:
                    emit_d_transposes(j // 4 - 1)
                    emit_d_stats(j // 4 - 1)
                emit_av(j, expT)
            first_d = 3 if overlap_d else 0
            for bb in range(first_d, NBANK):
                emit_d_copy(bb)
                emit_d_transposes(bb)
                emit_d_stats(bb)

            if debug:
                dq = dsb.tile([P, L], f32, name="dq", tag="dbgq")
                nc.vector.tensor_copy(dq[:], qT_sb[:])
                nc.sync.dma_start(out=dbg_d["qT"][:, :], in_=dq[:])
                dk = dsb.tile([P, L], f32, name="dk", tag="dbgk")
                nc.vector.tensor_copy(dk[:], kT_sb[:])
                nc.sync.dma_start(out=dbg_d["kT"][:, :], in_=dk[:])
                dv = dsb.tile([P, NCHUNK, D], f32, name="dv", tag="dbgv")
                nc.vector.tensor_copy(dv[:], v_sb[:])
                nc.sync.dma_start(out=dbgv_d[:, :, :], in_=dv[:])
                do = dsb.tile([P, L], f32, name="do", tag="dbgo")
                for bb in range(NBANK):
                    nc.vector.tensor_copy(
                        do[:, 512 * bb:512 * (bb + 1)],
                        obank[bb][:].rearrange("p c q -> p (c q)"))
                nc.sync.dma_start(out=dbg_d["outT"][:, :], in_=do[:])

    nc.compile()
    return nc


def _get_runner(cfg_key, consts):
    if cfg_key in _RUNNER_CACHE:
        return _RUNNER_CACHE[cfg_key]
    nc = _build_program(consts)
    _RUNNER_CACHE[cfg_key] = nc
    return nc


# ---------------------------------------------------------------- host logic
def _host_fixup_rows(out, value, mask, Wv, bv, sv):
    """Exactly reproduce reference for rows with no allowed keys."""
    for b in range(B):
        cnt = np.cumsum(~mask[b])
        rows = np.where(cnt == 0)[0]
        if rows.size == 0:
            continue
        x = value[b].astype(np.float32) @ Wv.T.astype(np.float32) + bv
        time = 1.0 / (1.0 + np.exp(-x[:, :1])) * np.exp(sv) + 1.1
        xn = x[:, 1:]
        s = (time * time - 1.0) / np.sum(xn * xn, axis=-1, keepdims=True)
        vproj = np.concatenate([time, xn * np.sqrt(s)], axis=-1)
        ave = vproj.mean(axis=0)
        lor = -ave[0] ** 2 + np.sum(ave[1:] ** 2)
        denom = np.sqrt(max(abs(lor), 1e-8))
        out[b, rows] = (ave / denom).astype(np.float32)


def _pack_wpad(Wq, Wk, Wv, pad01):
    from ml_dtypes import bfloat16
    wp = np.zeros((D, 3 * D + NCHUNK), dtype=bfloat16)
    wp[:, 0:D] = Wq.T.astype(bfloat16)
    wp[:, D:2 * D] = Wk.T.astype(bfloat16)
    wp[:, 2 * D:3 * D] = Wv.T.astype(bfloat16)
    return wp


def kernel(query, key, value, mask, Wq, bq, sq, Wk, bk, sk, Wv, bv, sv,
           attn_scale, attn_bias):
    from ml_dtypes import bfloat16
    from concourse.bass_utils import run_bass_kernel_spmd

    query = np.asarray(query, dtype=np.float32)
    key = np.asarray(key, dtype=np.float32)
    value = np.asarray(value, dtype=np.float32)
    mask = np.asarray(mask).astype(bool)
    Wq, Wk, Wv = (np.asarray(w, dtype=np.float32) for w in (Wq, Wk, Wv))
    bq, bk, bv = (np.asarray(b, dtype=np.float32).reshape(-1)
                  for b in (bq, bk, bv))

    has_bias = bool(np.any(bq) or np.any(bk) or np.any(bv))
    consts = dict(
        es_q=float(np.exp(np.float32(sq))),
        es_k=float(np.exp(np.float32(sk))),
        es_v=float(np.exp(np.float32(sv))),
        c1=float(2.0 / np.asarray(attn_scale, dtype=np.float32).reshape(-1)[0]),
        has_bias=has_bias,
    )
    cfg_key = tuple(sorted(consts.items()))
    nc = _get_runner(cfg_key, consts)

    pad01 = (~mask).astype(np.float32)
    wbase = np.zeros((D, 3 * D + NCHUNK), dtype=bfloat16)
    wbase[:, 0:D] = Wq.T.astype(bfloat16)
    wbase[:, D:2 * D] = Wk.T.astype(bfloat16)
    wbase[:, 2 * D:3 * D] = Wv.T.astype(bfloat16)
    in_maps = []
    for b in range(B):
        wp = wbase.copy()
        wp[:, 3 * D:] = pad01[b].reshape(NCHUNK, P).T.astype(bfloat16)
        m = {
            "q": np.ascontiguousarray(query[b].T).astype(bfloat16),
            "k": np.ascontiguousarray(key[b].T).astype(bfloat16),
            "v": np.ascontiguousarray(value[b].T).astype(bfloat16),
            "wpack": wp,
        }
        if has_bias:
            m["bq"] = bq.reshape(1, D)
            m["bk"] = bk.reshape(1, D)
            m["bv"] = bv.reshape(1, D)
        in_maps.append(m)

    res = run_bass_kernel_spmd(nc, in_maps, core_ids=list(range(B)))
    out = np.stack([res.results[b]["out"] for b in range(B)], axis=0)
    _host_fixup_rows(out, value, mask, Wv, bv, float(np.float32(sv)))
    return out


# revision 41
# speedup vs baseline: 1.2003x; 1.0202x over previous
"""Trainium2 Bass kernel for LorentzSelfAttention (B=8, L=2048, D=128, 1 head).

Sharding: data-parallel over batch — core b handles batch element b.

Per-core pipeline (L=2048, D=128, 16 row-chunks of 128, 4 groups of 4):
  Inputs arrive HOST-TRANSPOSED and bf16: xT [D, L] per tensor, loaded with
  ONE full-tensor DMA each on separate DMA queues (sync/scalar/gpsimd) so
  transfers overlap the framework preamble and each other. Weights wT for
  q/k/v plus the pad row are packed into a single [D, 3D+16] bf16 DMA.

  ONE ACT table (exp_and_others) for the whole kernel: sigmoid is computed
  as 0.5*tanh(x/2)+0.5 (tanh lives in the exp table), sqrt/rsqrt via DVE
  bit-trick + Newton (reciprocal), exp for attention. No mid-kernel
  ACT_TABLE_LOADs and no batched-stats sync point.

  Phase B (per group g, software-pipelined):
    12 bf16 matmuls (x-chunk stationary) -> PSUM [l, dout] f32; tanh of
    col 0 and Square+reduce of narrow cols read PSUM directly; per-group
    stats ([P, 12]) -> time / sqrt(s) via DVE Newton; narrow scaled
    PSUM->SBUF in one op (q/k: bf16, v: f32r with pad folded in); q/k
    chunks PE-transposed (bf16, 1 cyc/row) into qT/kT. Transposes of
    group g are emitted after group g+1's matmuls so the PE never waits
    on the stats chain.

  Phase C: scores transposed S_T[j, i] = <k_j, q_i>_L, bf16 matmuls in
    512-col slabs, exp (unnormalized — final Lorentz normalization is
    scale-invariant so softmax constants cancel) -> f32r expT; causal
    diag-block mask multiply on GpSimd; AV accumulates transposed in a
    4-bank PSUM tile outT_ps[d, i] via f32r matmuls (1 cyc/row).

  Phase D is folded INTO Phase C per PSUM bank: bank b of outT completes
    at j=4b+3, so its copy-out (GpSimd), PE transposes back to natural,
    Lorentz-norm stats (Square on GpSimd, reduce + rsqrt Newton on DVE)
    and the per-bank output DMA all overlap later j iterations.

Rows with an empty allowed key set (softmax over all -inf) are fixed up
exactly on host (a ~0-2 row prefix per batch).
"""

import numpy as np

B, L, D = 8, 2048, 128
P = 128
NCHUNK = L // P   # 16
G = 4             # chunks per group
NGROUP = NCHUNK // G  # 4
NBANK = 4         # 512-col PSUM banks of outT

_RUNNER_CACHE: dict = {}

MAGIC_SQRT = 0x1FBD1DF5


def _bcast3(bass, ap2, inner):
    """[P, n] AP -> [P, n, inner] broadcast view (step-0 innermost)."""
    return bass.AP(tensor=ap2.tensor, offset=ap2.offset,
                   ap=[ap2.ap[0], ap2.ap[1], [0, inner]])


# ---------------------------------------------------------------- device code
def _build_program(consts):
    from contextlib import ExitStack

    import concourse.bacc as bacc
    import concourse.bass as bass
    import concourse.mybir as mybir
    import concourse.tile as tile
    from concourse import masks

    f32 = mybir.dt.float32
    f32r = mybir.dt.float32r
    bf16 = mybir.dt.bfloat16
    i32 = mybir.dt.int32
    AF = mybir.ActivationFunctionType
    OP = mybir.AluOpType

    es = {"q": consts["es_q"], "k": consts["es_k"], "v": consts["es_v"]}
    c1 = consts["c1"]
    has_bias = consts["has_bias"]

    nc = bacc.Bacc("TRN2", target_bir_lowering=False, debug=False)

    xT_d = {}
    for nm in ("q", "k", "v"):
        xT_d[nm] = nc.dram_tensor(nm, [D, L], bf16, kind="ExternalInput").ap()
    # packed: wqT | wkT | wvT | pad(as [P, NCHUNK])
    wp_d = nc.dram_tensor("wpack", [D, 3 * D + NCHUNK], bf16,
                          kind="ExternalInput").ap()
    bias_d = {}
    if has_bias:
        for nm in ("q", "k", "v"):
            bias_d[nm] = nc.dram_tensor(f"b{nm}", [1, D], f32,
                                        kind="ExternalInput").ap()
    out_d = nc.dram_tensor("out", [L, D], f32, kind="ExternalOutput").ap()
    debug = consts.get("debug", False)
    if debug:
        dbg_d = {nm: nc.dram_tensor(f"dbg_{nm}", [D, L], f32,
                                    kind="ExternalOutput").ap()
                 for nm in ("qT", "kT", "outT")}
        dbgv_d = nc.dram_tensor("dbg_v", [P, NCHUNK, D], f32,
                                kind="ExternalOutput").ap()

    import os as _os
    TENSORS = ("q", "k", "v")

    with tile.TileContext(nc) as tc, ExitStack() as octx:
        cpool = octx.enter_context(tc.tile_pool(name="consts", bufs=1))

        # ---- inputs first: big DMAs on separate queues overlap preamble.
        # wpack is tiny and gates the first matmul -> first on the fast sync
        # queue; q in quarters so matmuls start after 1/4 of the transfer;
        # v (needed last) rides the slow gpsimd software queue.
        wpack = cpool.tile([P, 3 * D + NCHUNK], bf16)
        nc.sync.dma_start(out=wpack[:], in_=wp_d[:, :])
        xsb = {}
        for nm in TENSORS:
            xsb[nm] = cpool.tile([P, L], bf16, name=f"x_{nm}", tag=f"x_{nm}")
        Q4 = L // 4
        for qq in range(4):
            nc.sync.dma_start(out=xsb["q"][:, qq * Q4:(qq + 1) * Q4],
                              in_=xT_d["q"][:, qq * Q4:(qq + 1) * Q4])
        nc.scalar.dma_start(out=xsb["k"][:], in_=xT_d["k"][:, :])
        nc.gpsimd.dma_start(out=xsb["v"][:], in_=xT_d["v"][:, :])
        w_sb = {nm: wpack[:, ti * D:(ti + 1) * D]
                for ti, nm in enumerate(TENSORS)}
        pad_sb = wpack[:, 3 * D:3 * D + NCHUNK]   # 0/1 in bf16 (exact)
        bias_sb = {}
        if has_bias:
            for nm in TENSORS:
                bt = cpool.tile([P, D], f32, name=f"bias_{nm}",
                                tag=f"bias_{nm}")
                bd = bias_d[nm]
                nc.scalar.dma_start(out=bt[:], in_=bass.AP(
                    tensor=bd.tensor, offset=bd.offset, ap=[[0, P], bd.ap[1]]))
                bias_sb[nm] = bt

        ident = cpool.tile([P, P], bf16)
        masks.make_identity(nc, ident[:])
        identf = cpool.tile([P, P], f32)
        masks.make_identity(nc, identf[:])
        ut01 = cpool.tile([P, P], f32)
        masks.make_upper_triangular(nc, ut01[:], val=1.0, diag=True)

        # persistent activations
        qT_sb = cpool.tile([P, L], bf16)      # [d, l], time row negated
        kT_sb = cpool.tile([P, L], bf16)
        v_sb = cpool.tile([P, NCHUNK, D], f32r)  # [l%128, chunk, d], pad-zeroed

        # DVE sqrt: y = sqrt(x) via bit-trick seed + Newton (reciprocal).
        # 1 iteration: ~1e-3 rel err; 2 iterations: ~5e-7.
        def dve_sqrt(pool, x_ap, n, tag, iters=2):
            y = pool.tile([P, n], f32, name=f"sq_{tag}", tag=f"sq_{tag}")
            nc.vector.tensor_scalar(out=y[:].bitcast(i32),
                                    in0=x_ap.bitcast(i32), scalar1=1,
                                    scalar2=None, op0=OP.arith_shift_right)
            nc.vector.tensor_scalar(out=y[:].bitcast(i32),
                                    in0=y[:].bitcast(i32), scalar1=MAGIC_SQRT,
                                    scalar2=None, op0=OP.add)
            for it in range(iters):
                r = pool.tile([P, n], f32, name=f"r{it}_{tag}",
                              tag=f"r{it}_{tag}")
                nc.vector.reciprocal(r[:], y[:])
                nc.vector.scalar_tensor_tensor(
                    out=r[:], in0=x_ap, scalar=0.5, in1=r[:],
                    op0=OP.mult, op1=OP.mult)
                nc.vector.scalar_tensor_tensor(
                    out=y[:], in0=y[:], scalar=0.5, in1=r[:],
                    op0=OP.mult, op1=OP.add)
            return y

        # ---------------- Phase B: projections, per-group pipeline ----------
        with ExitStack() as ctxB:
            ps_l = ctxB.enter_context(
                tc.tile_pool(name="ps_l", bufs=2, space="PSUM"))
            ps_q = ctxB.enter_context(
                tc.tile_pool(name="ps_q", bufs=1, space="PSUM"))
            misc = ctxB.enter_context(tc.tile_pool(name="misc", bufs=2))
            stat = ctxB.enter_context(tc.tile_pool(name="stat", bufs=2))
            qknat = ctxB.enter_context(tc.tile_pool(name="qknat", bufs=2))

            lin_g = {}      # g -> {nm: psum tile}
            sqs_g = {}      # g -> [P, 12] sqrt(s), v cols pad-folded
            time_g = {}     # g -> [P, 12] time, v cols pad-folded
            nat_g = {}      # g -> {nm: scaled natural bf16 chunk (q/k only)}

            def emit_mm_stats(g):
                lin_g[g] = {}
                tg = stat.tile([P, 3 * G], f32, name=f"tg{g}", tag="tg")
                ssg = stat.tile([P, 3 * G], f32, name=f"ssg{g}", tag="ssg")
                for ti, nm in enumerate(TENSORS):
                    lin4 = ps_l.tile([P, G * D], f32, tag=f"lin_{nm}")
                    lin_g[g][nm] = lin4
                    for c in range(G):
                        nc.tensor.matmul(
                            lin4[:, c * D:(c + 1) * D],
                            xsb[nm][:, (g * G + c) * P:(g * G + c + 1) * P],
                            w_sb[nm], start=True, stop=True)
                    if has_bias:
                        nc.vector.tensor_add(
                            lin4[:], lin4[:],
                            bass.AP(tensor=bias_sb[nm].tensor,
                                    offset=bias_sb[nm][:].offset,
                                    ap=[bias_sb[nm][:].ap[0], [0, G], [1, D]]))
                    src4 = lin4[:].rearrange("p (c d) -> p c d", d=D)
                    # tanh(x/2) -> sigmoid pieces (exp-table resident)
                    nc.scalar.activation(
                        tg[:, ti * G:(ti + 1) * G], src4[:, :, 0:1],
                        AF.Tanh, scale=0.5)
                    # time = es*sigmoid + 1.1 = (es/2)*tanh + (es/2 + 1.1)
                    # (emitted before the reduce so it's off the critical
                    # post-reduce chain)
                    e2 = es[nm] * 0.5
                    nc.vector.tensor_scalar(
                        out=tg[:, ti * G:(ti + 1) * G],
                        in0=tg[:, ti * G:(ti + 1) * G],
                        scalar1=e2, scalar2=e2 + 1.1,
                        op0=OP.mult, op1=OP.add)
                    sq4 = misc.tile([P, G, D - 1], bf16, name=f"sq4{nm}",
                                    tag=f"sq4_{nm}")
                    nc.scalar.activation(sq4[:], src4[:, :, 1:D], AF.Square)
                    nc.vector.tensor_reduce(
                        ssg[:, ti * G:(ti + 1) * G], sq4[:],
                        mybir.AxisListType.X, OP.add)
                # s = (time^2 - 1) / ssq ; sqs = sqrt(s)
                sval = stat.tile([P, 3 * G], f32, name=f"sval{g}", tag="sval")
                nc.vector.tensor_mul(sval[:], tg[:], tg[:])
                inv = stat.tile([P, 3 * G], f32, name=f"inv{g}", tag="inv")
                nc.vector.reciprocal(inv[:], ssg[:])
                nc.vector.scalar_tensor_tensor(
                    out=sval[:], in0=sval[:], scalar=-1.0, in1=inv[:],
                    op0=OP.add, op1=OP.mult)
                sqs = dve_sqrt(stat, sval[:], 3 * G, f"b{g}", iters=1)
                # fold pad zeroing into v scale (time handled in its write)
                vb = 2 * G
                nc.vector.tensor_mul(sqs[:, vb:vb + G], sqs[:, vb:vb + G],
                                     pad_sb[:, g * G:(g + 1) * G])
                sqs_g[g] = sqs
                time_g[g] = tg
                # scale narrow PSUM -> SBUF dest; write time col
                nat_g[g] = {}
                for ti, nm in enumerate(TENSORS):
                    src4 = lin_g[g][nm][:].rearrange("p (c d) -> p c d", d=D)
                    if nm == "v":
                        dst = v_sb[:, g * G:(g + 1) * G, :]
                        # time col with pad fold in one op
                        nc.vector.tensor_mul(
                            dst[:, :, 0:1], tg[:, vb:vb + G],
                            pad_sb[:, g * G:(g + 1) * G])
                        # narrow scale on ACT: per-chunk Copy with
                        # per-partition scale (keeps DVE off the hot path)
                        for c in range(G):
                            nc.scalar.activation(
                                dst[:, c, 1:D], src4[:, c, 1:D], AF.Copy,
                                scale=sqs[:, vb + c:vb + c + 1])
                        continue
                    nat = qknat.tile([P, G, D], bf16, name=f"nat{nm}{g}",
                                     tag=f"nat_{nm}")
                    nat_g[g][nm] = nat
                    dst = nat[:]
                    tsign = -1.0 if nm == "q" else 1.0
                    nc.vector.tensor_scalar(
                        out=dst[:, :, 0:1], in0=tg[:, ti * G:(ti + 1) * G],
                        scalar1=tsign, scalar2=0.0, op0=OP.mult, op1=OP.add)
                    nc.vector.tensor_mul(
                        dst[:, :, 1:D], src4[:, :, 1:D],
                        _bcast3(bass, sqs[:, ti * G:(ti + 1) * G], D - 1))

            def emit_transposes(g):
                for nm in ("q", "k"):
                    qkT4 = ps_q.tile([P, G * P], bf16, tag=f"qkT_{nm}")
                    nat = nat_g[g][nm]
                    for c in range(G):
                        nc.tensor.transpose(
                            qkT4[:, c * P:(c + 1) * P], nat[:, c, :], ident[:])
                    dst = qT_sb if nm == "q" else kT_sb
                    nc.vector.tensor_copy(
                        dst[:, g * G * P:(g + 1) * G * P], qkT4[:])

            for g in range(NGROUP):
                emit_mm_stats(g)
                if g >= 1:
                    emit_transposes(g - 1)
            emit_transposes(NGROUP - 1)

        # ---------------- Phase C + per-bank Phase D ----------
        with ExitStack() as ctxC:
            ps_s = ctxC.enter_context(
                tc.tile_pool(name="ps_s", bufs=2, space="PSUM"))
            ps_o = ctxC.enter_context(
                tc.tile_pool(name="ps_o", bufs=1, space="PSUM"))
            ps_d = ctxC.enter_context(
                tc.tile_pool(name="ps_d", bufs=2, space="PSUM"))
            sb_e = ctxC.enter_context(tc.tile_pool(name="sb_e", bufs=2))
            dsb = ctxC.enter_context(tc.tile_pool(name="dsb", bufs=2))
            dstat = ctxC.enter_context(tc.tile_pool(name="dstat", bufs=2))

            outT_ps = ps_o.tile([P, L], f32)
            obank = {}   # b -> sbuf copy of finished bank
            otr = {}     # b -> natural transposed PSUM tile

            def emit_qk_exp(j):
                ncols = (NCHUNK - j) * P
                base = j * P
                expT = sb_e.tile([P, L], f32r, tag="expT")
                kblk = kT_sb[:, base:base + P]
                ofs = 0
                while ofs < ncols:
                    sw = min(512, ncols - ofs)
                    s_ps = ps_s.tile([P, 512], f32, tag="s")
                    nc.tensor.matmul(
                        s_ps[:, :sw], kblk,
                        qT_sb[:, base + ofs:base + ofs + sw],
                        start=True, stop=True)
                    nc.scalar.activation(
                        expT[:, ofs:ofs + sw], s_ps[:, :sw], AF.Exp, scale=c1)
                    ofs += sw
                # causal mask inside the diagonal block (gpsimd; writes f32r)
                if _os.environ.get("LK_UT01", "gpsimd") == "gpsimd":
                    nc.gpsimd.tensor_mul(expT[:, 0:P], expT[:, 0:P], ut01[:])
                else:
                    nc.vector.tensor_mul(expT[:, 0:P], expT[:, 0:P], ut01[:])
                return expT

            def emit_av(j, expT):
                base = j * P
                col = base
                while col < L:
                    bank_end = min(L, (col // 512 + 1) * 512)
                    kbank = bank_end // 512 - 1
                    last_j = 4 * kbank + 3
                    nc.tensor.matmul(
                        outT_ps[:, col:bank_end],
                        v_sb[:, j, :],
                        expT[:, col - base:bank_end - base],
                        start=(j == 0), stop=(j == last_j))
                    col = bank_end

            def emit_d_copy(b):
                ob = dsb.tile([P, NBANK, P], f32, name=f"ob{b}", tag="obank")
                obank[b] = ob
                src = outT_ps[:, 512 * b:512 * (b + 1)].rearrange(
                    "p (c q) -> p c q", q=P)
                if b == NBANK - 1:
                    # tail bank: ACT is idle by now (exp done), DVE is not
                    nc.scalar.activation(ob[:], src, AF.Copy)
                else:
                    nc.vector.tensor_copy(ob[:], src)

            def emit_d_transposes(b):
                o_ps4 = ps_d.tile([P, NBANK, P], f32, tag="otr")
                otr[b] = o_ps4
                for c in range(NBANK):
                    nc.tensor.transpose(
                        o_ps4[:, c, :], obank[b][:, c, :], identf[:])

            def emit_d_stats(b):
                o_ps4 = otr[b]
                scr = dsb.tile([P, NBANK, P], f32, name=f"scr{b}", tag="scr")
                na = dstat.tile([P, NBANK], f32, name=f"na{b}", tag="na")
                nc.scalar.activation(scr[:], o_ps4[:], AF.Square)
                nc.vector.tensor_reduce(na[:], scr[:], mybir.AxisListType.X,
                                        OP.add)
                tt = dstat.tile([P, NBANK], f32, name=f"tt{b}", tag="tt")
                nc.vector.tensor_scalar(
                    out=tt[:], in0=scr[:, :, 0:1], scalar1=2.0, scalar2=0.0,
                    op0=OP.mult, op1=OP.add)
                nc.vector.tensor_sub(na[:], tt[:], na[:])
                sqna = dve_sqrt(dstat, na[:], NBANK, f"d{b}")
                rn = dstat.tile([P, NBANK], f32, name=f"rn{b}", tag="rn")
                nc.vector.reciprocal(rn[:], sqna[:])
                osb = dsb.tile([P, NBANK, P], f32, name=f"osb{b}", tag="osb")
                nc.vector.tensor_mul(osb[:], o_ps4[:],
                                     _bcast3(bass, rn[:], P))
                nc.sync.dma_start(
                    out=out_d[b * 512:(b + 1) * 512, :].rearrange(
                        "(c p) d -> p c d", p=P),
                    in_=osb[:])

            overlap_d = _os.environ.get("LK_DOVERLAP", "1") == "1"
            for j in range(NCHUNK):
                if overlap_d and j > 0 and j % 4 == 0:
                    emit_d_copy(j // 4 - 1)
                expT = emit_qk_exp(j)
                if overlap_d and j > 0 and j % 4 == 0:
                    emit_d_transposes(j // 4 - 1)
                    emit_d_stats(j // 4 - 1)
                emit_av(j, expT)
            first_d = 3 if overlap_d else 0
            for bb in range(first_d, NBANK):
                emit_d_copy(bb)
                emit_d_transposes(bb)
                emit_d_stats(bb)

            if debug:
                dq = dsb.tile([P, L], f32, name="dq", tag="dbgq")
                nc.vector.tensor_copy(dq[:], qT_sb[:])
                nc.sync.dma_start(out=dbg_d["qT"][:, :], in_=dq[:])
                dk = dsb.tile([P, L], f32, name="dk", tag="dbgk")
                nc.vector.tensor_copy(dk[:], kT_sb[:])
                nc.sync.dma_start(out=dbg_d["kT"][:, :], in_=dk[:])
                dv = dsb.tile([P, NCHUNK, D], f32, name="dv", tag="dbgv")
                nc.vector.tensor_copy(dv[:], v_sb[:])
                nc.sync.dma_start(out=dbgv_d[:, :, :], in_=dv[:])
                do = dsb.tile([P, L], f32, name="do", tag="dbgo")
                for bb in range(NBANK):
                    nc.vector.tensor_copy(
                        do[:, 512 * bb:512 * (bb + 1)],
                        obank[bb][:].rearrange("p c q -> p (c q)"))
                nc.sync.dma_start(out=dbg_d["outT"][:, :], in_=do[:])

    nc.compile()
    return nc


def _get_runner(cfg_key, consts):
    if cfg_key in _RUNNER_CACHE:
        return _RUNNER_CACHE[cfg_key]
    nc = _build_program(consts)
    _RUNNER_CACHE[cfg_key] = nc
    return nc


# ---------------------------------------------------------------- host logic
def _host_fixup_rows(out, value, mask, Wv, bv, sv):
    """Exactly reproduce reference for rows with no allowed keys."""
    for b in range(B):
        cnt = np.cumsum(~mask[b])
        rows = np.where(cnt == 0)[0]
        if rows.size == 0:
            continue
        x = value[b].astype(np.float32) @ Wv.T.astype(np.float32) + bv
        time = 1.0 / (1.0 + np.exp(-x[:, :1])) * np.exp(sv) + 1.1
        xn = x[:, 1:]
        s = (time * time - 1.0) / np.sum(xn * xn, axis=-1, keepdims=True)
        vproj = np.concatenate([time, xn * np.sqrt(s)], axis=-1)
        ave = vproj.mean(axis=0)
        lor = -ave[0] ** 2 + np.sum(ave[1:] ** 2)
        denom = np.sqrt(max(abs(lor), 1e-8))
        out[b, rows] = (ave / denom).astype(np.float32)


def _pack_wpad(Wq, Wk, Wv, pad01):
    from ml_dtypes import bfloat16
    wp = np.zeros((D, 3 * D + NCHUNK), dtype=bfloat16)
    wp[:, 0:D] = Wq.T.astype(bfloat16)
    wp[:, D:2 * D] = Wk.T.astype(bfloat16)
    wp[:, 2 * D:3 * D] = Wv.T.astype(bfloat16)
    return wp


def kernel(query, key, value, mask, Wq, bq, sq, Wk, bk, sk, Wv, bv, sv,
           attn_scale, attn_bias):
    from ml_dtypes import bfloat16
    from concourse.bass_utils import run_bass_kernel_spmd

    query = np.asarray(query, dtype=np.float32)
    key = np.asarray(key, dtype=np.float32)
    value = np.asarray(value, dtype=np.float32)
    mask = np.asarray(mask).astype(bool)
    Wq, Wk, Wv = (np.asarray(w, dtype=np.float32) for w in (Wq, Wk, Wv))
    bq, bk, bv = (np.asarray(b, dtype=np.float32).reshape(-1)
                  for b in (bq, bk, bv))

    has_bias = bool(np.any(bq) or np.any(bk) or np.any(bv))
    consts = dict(
        es_q=float(np.exp(np.float32(sq))),
        es_k=float(np.exp(np.float32(sk))),
        es_v=float(np.exp(np.float32(sv))),
        c1=float(2.0 / np.asarray(attn_scale, dtype=np.float32).reshape(-1)[0]),
        has_bias=has_bias,
    )
    cfg_key = tuple(sorted(consts.items()))
    nc = _get_runner(cfg_key, consts)

    pad01 = (~mask).astype(np.float32)
    wbase = np.zeros((D, 3 * D + NCHUNK), dtype=bfloat16)
    wbase[:, 0:D] = Wq.T.astype(bfloat16)
    wbase[:, D:2 * D] = Wk.T.astype(bfloat16)
    wbase[:, 2 * D:3 * D] = Wv.T.astype(bfloat16)
    in_maps = []
    for b in range(B):
        wp = wbase.copy()
        wp[:, 3 * D:] = pad01[b].reshape(NCHUNK, P).T.astype(bfloat16)
        m = {
            "q": np.ascontiguousarray(query[b].T).astype(bfloat16),
            "k": np.ascontiguousarray(key[b].T).astype(bfloat16),
            "v": np.ascontiguousarray(value[b].T).astype(bfloat16),
            "wpack": wp,
        }
        if has_bias:
            m["bq"] = bq.reshape(1, D)
            m["bk"] = bk.reshape(1, D)
            m["bv"] = bv.reshape(1, D)
        in_maps.append(m)

    res = run_bass_kernel_spmd(nc, in_maps, core_ids=list(range(B)))
    out = np.stack([res.results[b]["out"] for b in range(B)], axis=0)
    _host_fixup_rows(out, value, mask, Wv, bv, float(np.float32(sv)))
    return out
